# revision 1
# baseline (speedup 1.0000x reference)
"""Llama trunk (2 layers, before final norm) on 8 trn2 cores.

Sharding: Megatron tensor-parallel over 8 cores.
  - attention: 4 q-heads + 1 kv-head per core (GQA group stays local)
  - MLP: 1024 of 8192 intermediate dims per core
  - residual stream xt kept in fp16, transposed [DM(part), S(free)]
Key implementation points vs the bf16/f32 baseline:
  - fp16 everywhere off the PE accumulators: weights, activations,
    residual, and the AllReduce payloads (halves collective bytes, and
    fp16 enables the DVE 4x elementwise mode).
  - x is pre-normalized once per norm (xbn = x*r) so no post-matmul
    rescale of q/k/v or gate/up psums is needed; v comes out of the
    same qkv matmul pass and is PE-transposed into natural layout.
  - 1/rms broadcast across partitions via a K=1 outer-product matmul
    instead of a DRAM bounce; softmax key-sums via a ones-stationary
    matmul; causal mask accumulated into the scores psum by the PE
    (-30000*I x step-matrix matmul) so DVE/ACT never touch the mask.
  - layer-0 attn norm (r and x*r) is computed on the host.
  - weight/staging DMAs are batched (4 k-tiles per transfer).
  - final down-proj partials are summed on the host (saves the last
    AllReduce); the 3 on-device AllReduces run in fp16 DM-halves.
"""
import math
from contextlib import ExitStack

import numpy as np

import concourse.bass as bass
import concourse.tile as tile
from concourse import bacc, mybir
from concourse.alu_op_type import AluOpType
from concourse.bass_utils import run_bass_kernel_spmd

L, H, KVH, D = 2, 32, 8, 64
DM, FF = 2048, 8192
B, S = 1, 1024
EPS, THETA = 1e-5, 10000.0
NCORES = 8
QH = H // NCORES            # 4 q heads per core
QC = QH * D                 # 256 q cols per core
FFS = FF // NCORES          # 1024 ff dims per core
KT = DM // 128              # 16 contraction tiles over DM
FMT = FFS // 128            # 8 f tiles
NEGBIG = -30000.0

F32 = mybir.dt.float32
F16 = mybir.dt.float16
AF = mybir.ActivationFunctionType


def build(reps=1, debug_stage=None, skip_cc=False):
    nc = bacc.Bacc(None, target_bir_lowering=False, debug=False, num_devices=NCORES)
    xt_in = nc.dram_tensor("xt16", [DM, S], F16, kind="ExternalInput").ap()
    xbn0_in = nc.dram_tensor("xbn0", [DM, S], F16, kind="ExternalInput").ap()
    wqkv_in = nc.dram_tensor("wqkv", [L, DM, 384], F16, kind="ExternalInput").ap()
    wo_in = nc.dram_tensor("wo", [L, QC, DM], F16, kind="ExternalInput").ap()
    wg_in = nc.dram_tensor("wg", [L, DM, FFS], F16, kind="ExternalInput").ap()
    wu_in = nc.dram_tensor("wu", [L, DM, FFS], F16, kind="ExternalInput").ap()
    wd_in = nc.dram_tensor("wd", [L, FFS, DM], F16, kind="ExternalInput").ap()
    cos_in = nc.dram_tensor("cosr", [128, S], F16, kind="ExternalInput").ap()
    sin_in = nc.dram_tensor("sinr", [128, S], F16, kind="ExternalInput").ap()
    step_in = nc.dram_tensor("step", [2, 128, 256], F16, kind="ExternalInput").ap()
    nbi_in = nc.dram_tensor("negbigI", [128, 128], F16, kind="ExternalInput").ap()
    id_in = nc.dram_tensor("ident", [128, 64], F16, kind="ExternalInput").ap()
    y_out = nc.dram_tensor("y", [DM, S], F16, kind="ExternalOutput").ap()

    with tile.TileContext(nc) as tc, ExitStack() as ctx, \
            nc.allow_low_precision(reason="deliberate fp16 pipeline, tol 2e-2"):
        const = ctx.enter_context(tc.tile_pool(name="const", bufs=1))
        xtp = ctx.enter_context(tc.tile_pool(name="xtp", bufs=1))
        wpool = ctx.enter_context(tc.tile_pool(name="wpool", bufs=2))
        wob = ctx.enter_context(tc.tile_pool(name="wob", bufs=1))
        wbig = ctx.enter_context(tc.tile_pool(name="wbig", bufs=2))
        sq = ctx.enter_context(tc.tile_pool(name="sq", bufs=2))
        rp = ctx.enter_context(tc.tile_pool(name="rp", bufs=1))
        attn_sb = ctx.enter_context(tc.tile_pool(name="attn_sb", bufs=1))
        ropet = ctx.enter_context(tc.tile_pool(name="ropet", bufs=1))
        vap = ctx.enter_context(tc.tile_pool(name="vap", bufs=1))
        expp = ctx.enter_context(tc.tile_pool(name="expp", bufs=2))
        stkp = ctx.enter_context(tc.tile_pool(name="stkp", bufs=1))
        evp = ctx.enter_context(tc.tile_pool(name="evp", bufs=4))
        stage = ctx.enter_context(tc.tile_pool(name="stage", bufs=2))
        arp = ctx.enter_context(tc.tile_pool(name="arp", bufs=2))
        actp = ctx.enter_context(tc.tile_pool(name="actp", bufs=1))

        dram = ctx.enter_context(tc.tile_pool(name="dram", bufs=2, space="DRAM"))
        ccp = ctx.enter_context(tc.tile_pool(name="ccp", bufs=2, space="DRAM"))

        # ---- persistent constants ----
        onesb = const.tile([128, 1], F16)
        nc.vector.memset(onesb[:], 1.0)
        onesr = const.tile([1, 128], F16)
        nc.vector.memset(onesr[:], 1.0)
        cosr = const.tile([128, S], F16)
        nc.sync.dma_start(cosr[:], cos_in[:])
        sinr = const.tile([128, S], F16)
        nc.sync.dma_start(sinr[:], sin_in[:])
        step0 = const.tile([128, 256], F16)
        nc.sync.dma_start(step0[:], step_in[0, :, :])
        step1 = const.tile([128, 256], F16)
        nc.sync.dma_start(step1[:], step_in[1, :, :])
        nbi = const.tile([128, 128], F16)
        nc.sync.dma_start(nbi[:], nbi_in[:])
        ident = const.tile([128, 64], F16)
        nc.sync.dma_start(ident[:], id_in[:])
        epsb = const.tile([1, 1], F32)
        nc.vector.memset(epsb[:], EPS)

        # residual stream + normalized copy, fp16, resident (big tiles)
        xtb = xtp.tile([128, KT * S], F16, tag="xtb", name="xtb")
        xbnb = xtp.tile([128, KT * S], F16, tag="xbnb", name="xbnb")

        def XT(k):
            return xtb[:, k * S:(k + 1) * S]

        def XBN(k):
            return xbnb[:, k * S:(k + 1) * S]

        def load_x():
            for dst, src in ((xtb, xt_in), (xbnb, xbn0_in)):
                nc.sync.dma_start(
                    dst[:],
                    bass.AP(tensor=src.tensor, offset=src.offset,
                            ap=[[S, 128], [128 * S, KT], [1, S]]))

        def norm_stats_start(ctx_norm):
            ps_stat = ctx_norm.enter_context(tc.tile_pool(name="ps_stat", bufs=1, space="PSUM"))
            return [ps_stat.tile([1, 512], F32, tag=f"ssum{st}", name=f"ssum{st}")
                    for st in range(2)]

        def norm_sq_and_acc(ssum, k, first, last):
            xsq = sq.tile([128, S], F16, tag="xsq")
            nc.vector.tensor_tensor(xsq[:], XT(k), XT(k), AluOpType.mult)
            for st in range(2):
                nc.tensor.matmul(ssum[st][:], onesb[:], xsq[:, st * 512:(st + 1) * 512],
                                 start=first, stop=last)

        def norm_finish(ssum, ctx_norm):
            """r = rsqrt(mean+eps); broadcast via outer-product; xbn = xt*r"""
            rr16 = rp.tile([1, S], F16, tag="rr16")
            for st in range(2):
                rs = rp.tile([1, 512], F32, tag="rs")
                nc.scalar.activation(rs[:], ssum[st][:], AF.Sqrt,
                                     bias=epsb[:], scale=1.0 / DM)
                rrf = rp.tile([1, 512], F32, tag="rrf")
                nc.vector.reciprocal(rrf[:], rs[:])
                nc.vector.tensor_copy(rr16[:, st * 512:(st + 1) * 512], rrf[:])
            ps_rb = ctx_norm.enter_context(tc.tile_pool(name="ps_rb", bufs=1, space="PSUM"))
            rb_ps = ps_rb.tile([128, S], F32, tag="rb_ps")
            for st in range(2):
                sl = slice(st * 512, (st + 1) * 512)
                nc.tensor.matmul(rb_ps[:, sl], onesr[:], rr16[:, sl], start=True, stop=True)
            rb = rp.tile([128, S], F16, tag="rb")
            nc.vector.tensor_copy(rb[:], rb_ps[:])
            for k in range(KT):
                nc.vector.tensor_tensor(XBN(k), XT(k), rb[:], AluOpType.mult)

        def residual_and_norm(cc_out, skip_norm=False):
            """xt += AR result (fp16); then sumsq stats + xbn for next block."""
            ctx_norm = ExitStack()
            ssum = None if skip_norm else norm_stats_start(ctx_norm)
            for q in range(4):
                ar_w = arp.tile([128, 4 * S], F16, tag="ar")
                nc.sync.dma_start(
                    ar_w[:],
                    bass.AP(tensor=cc_out.tensor, offset=cc_out.offset + q * 4 * 128 * S,
                            ap=[[S, 128], [128 * S, 4], [1, S]]))
                for i in range(4):
                    k = q * 4 + i
                    nc.vector.tensor_add(XT(k), XT(k), ar_w[:, i * S:(i + 1) * S])
                    if not skip_norm:
                        norm_sq_and_acc(ssum, k, first=(k == 0), last=(k == KT - 1))
            if not skip_norm:
                norm_finish(ssum, ctx_norm)
            ctx_norm.close()

        def attn_block(l):
            # ---- qkv matmuls on pre-normalized xbn ----
            ctx_qkv = ExitStack()
            ps_qkv = ctx_qkv.enter_context(tc.tile_pool(name="ps_qkv", bufs=1, space="PSUM"))
            pq = [[ps_qkv.tile([128, 512], F32, tag=f"pq{m}_{st}", name=f"pq{m}_{st}")
                   for st in range(2)] for m in range(3)]
            for g in range(4):
                wt4 = wpool.tile([128, 4 * 384], F16, tag="wqkv")
                nc.sync.dma_start(
                    wt4[:],
                    bass.AP(tensor=wqkv_in.tensor,
                            offset=wqkv_in.offset + l * DM * 384 + g * 4 * 128 * 384,
                            ap=[[384, 128], [128 * 384, 4], [1, 384]]))
                for i in range(4):
                    k = g * 4 + i
                    w0 = i * 384
                    st_, sp_ = (k == 0), (k == KT - 1)
                    for st in range(2):
                        sl = slice(st * 512, (st + 1) * 512)
                        nc.tensor.matmul(pq[0][st][:], wt4[:, w0:w0 + 128], XBN(k)[:, sl],
                                         start=st_, stop=sp_)
                        nc.tensor.matmul(pq[1][st][:], wt4[:, w0 + 128:w0 + 256], XBN(k)[:, sl],
                                         start=st_, stop=sp_)
                        nc.tensor.matmul(pq[2][st][:], wt4[:, w0 + 256:w0 + 384], XBN(k)[:, sl],
                                         start=st_, stop=sp_)
            # evacuate q (DVE) and kv (ACT) to fp16 sbuf
            q01 = attn_sb.tile([128, S], F16, tag="q01")
            q23 = attn_sb.tile([128, S], F16, tag="q23")
            kv2 = attn_sb.tile([128, S], F16, tag="kv2")
            for st in range(2):
                sl = slice(st * 512, (st + 1) * 512)
                nc.vector.tensor_copy(q01[:, sl], pq[0][st][:])
                nc.vector.tensor_copy(q23[:, sl], pq[1][st][:])
                nc.scalar.copy(kv2[:, sl], pq[2][st][:])
            ctx_qkv.close()

            # ---- v -> natural layout via PE transpose ----
            ctx_v = ExitStack()
            ps_v = ctx_v.enter_context(tc.tile_pool(name="ps_v", bufs=4, space="PSUM"))
            va = []
            for sj in range(8):
                tp = ps_v.tile([128, 64], F16, tag="tp")
                nc.tensor.transpose(tp[:], kv2[64:128, sj * 128:(sj + 1) * 128], ident[64:128, :])
                v = vap.tile([128, 65], F16, tag=f"va{sj}", name=f"va{sj}")
                nc.vector.tensor_copy(v[:, 0:64], tp[:])
                nc.vector.memset(v[:, 64:65], 1.0)
                va.append(v)
            ctx_v.close()

            # ---- RoPE on q01, q23 and k (kv2 rows 0:64 -> kt2) ----
            kt2 = attn_sb.tile([128, S], F16, tag="kt2")

            def rope(t, nrows, out, outrows):
                rot = ropet.tile([128, S], F16, tag="rot")
                for h0 in range(0, nrows, 64):
                    nc.scalar.dma_start(rot[h0:h0 + 32, :], t[h0 + 32:h0 + 64, :])
                    nc.scalar.dma_start(rot[h0 + 32:h0 + 64, :], t[h0:h0 + 32, :])
                t1 = ropet.tile([128, S], F16, tag="t1")
                nc.vector.tensor_tensor(t1[0:nrows, :], t[0:nrows, :], cosr[0:nrows, :],
                                        AluOpType.mult)
                t2 = ropet.tile([128, S], F16, tag="t2")
                nc.vector.tensor_tensor(t2[0:nrows, :], rot[0:nrows, :], sinr[0:nrows, :],
                                        AluOpType.mult)
                nc.vector.tensor_add(out[outrows, :], t1[0:nrows, :], t2[0:nrows, :])
            rope(q01, 128, q01, slice(0, 128))
            rope(q23, 128, q23, slice(0, 128))
            rope(kv2, 64, kt2, slice(0, 64))
            nc.scalar.dma_start(kt2[64:128, :], kt2[0:64, :])  # duplicate kv head

            # ---- attention (4 heads; j in pairs; sums via ones-matmul) ----
            stk0 = stkp.tile([128, S], F16, tag="stk0")
            stk1 = stkp.tile([128, S], F16, tag="stk1")
            sinv_sb = stkp.tile([1, 4 * S], F16, tag="sinv_sb")
            sinv_dram = dram.tile([4, S], F16, tag="sinv")
            ctx_att = ExitStack()
            ps_sc = ctx_att.enter_context(tc.tile_pool(name="ps_sc", bufs=2, space="PSUM"))
            ps_at = ctx_att.enter_context(tc.tile_pool(name="ps_at", bufs=1, space="PSUM"))
            ps_sm = ctx_att.enter_context(tc.tile_pool(name="ps_sm", bufs=1, space="PSUM"))
            # head pairs (parity, parity+2) share kv rows and va (GQA), so
            # each stationary (kt2 block / va / nbi) is loaded once and used
            # for both heads' matmuls. it-outer keeps psum accumulation
            # groups bank-exclusive (start=True clears a whole bank's
            # has_written bits).
            for parity in range(2):
                rows = slice(64 * parity, 64 * parity + 64)
                odd = parity == 1
                heads = (parity, parity + 2)     # -> (stk0, stk1)
                for it in range(4):
                    isl = slice(it * 256, (it + 1) * 256)
                    aps = [ps_at.tile([128, 512], F32, tag=f"aps{m}", name=f"aps{m}")
                           for m in range(2)]
                    if odd:
                        sums = [ps_sm.tile([1, 512], F32, tag=f"sums{m}", name=f"sums{m}")
                                for m in range(2)]
                    for jp in range(it + 1):
                        diag = jp == it
                        sps = [ps_sc.tile([128, 512], F32, tag=f"sps{m}", name=f"sps{m}")
                               for m in range(2)]
                        for half in range(2):
                            j = 2 * jp + half
                            ssl = slice(half * 256, half * 256 + 256)
                            if diag:
                                for m in range(2):
                                    nc.tensor.matmul(sps[m][:, ssl], nbi[:],
                                                     (step0, step1)[half][:],
                                                     start=True, stop=False)
                            for m, qt in enumerate((q01, q23)):
                                nc.tensor.matmul(sps[m][:, ssl],
                                                 kt2[rows, j * 128:(j + 1) * 128],
                                                 qt[rows, isl],
                                                 start=not diag, stop=True)
                        es = []
                        for m in range(2):
                            e = expp.tile([128, 512], F16, tag=f"e{m}", name=f"e{m}")
                            nc.scalar.activation(e[:], sps[m][:], AF.Exp)
                            es.append(e)
                        first, last = (jp == 0), (jp == it)
                        for half in range(2):
                            esl = slice(half * 256, half * 256 + 256)
                            vsl = slice(0, 64) if odd else slice(0, 65)
                            orows = slice(64, 128) if odd else slice(0, 65)
                            for m in range(2):
                                nc.tensor.matmul(aps[m][orows, 0:256],
                                                 va[2 * jp + half][:, vsl], es[m][:, esl],
                                                 start=(first and half == 0),
                                                 stop=(last and half == 1))
                            if odd:
                                for m in range(2):
                                    nc.tensor.matmul(sums[m][0:1, 0:256],
                                                     onesb[:], es[m][:, esl],
                                                     start=(first and half == 0),
                                                     stop=(last and half == 1))
                    arows = slice(64, 128) if odd else slice(0, 64)
                    for m in range(2):
                        h = heads[m]
                        srow = sums[m][0:1, 0:256] if odd else aps[m][64:65, 0:256]
                        nc.vector.reciprocal(
                            sinv_sb[0:1, h * S + it * 256:h * S + it * 256 + 256], srow)
                        nc.vector.tensor_copy((stk0, stk1)[m][arows, isl],
                                              aps[m][arows, 0:256])
            ctx_att.close()
            nc.scalar.dma_start(sinv_dram[:], sinv_sb[:])
            # normalize: stk *= 1/sums (broadcast per 64-row head block)
            for t, h0, h1 in ((stk0, 0, 1), (stk1, 2, 3)):
                sb = rp.tile([128, S], F16, tag="sinvb")
                nc.sync.dma_start(
                    sb[0:64, :],
                    bass.AP(tensor=sinv_dram.tensor, offset=sinv_dram.offset + h0 * S,
                            ap=[[0, 64], [1, S]]))
                nc.sync.dma_start(
                    sb[64:128, :],
                    bass.AP(tensor=sinv_dram.tensor, offset=sinv_dram.offset + h1 * S,
                            ap=[[0, 64], [1, S]]))
                nc.vector.tensor_tensor(t[:], t[:], sb[:], AluOpType.mult)

            if debug_stage == f"stk{l}":
                for ti, t in enumerate((stk0, stk1)):
                    nc.sync.dma_start(y_out[ti * 128:(ti + 1) * 128, :], t[:])
                for sj in range(8):
                    nc.sync.dma_start(y_out[256 + sj * 128:256 + (sj + 1) * 128, 0:65],
                                      va[sj][:])
                nc.sync.dma_start(y_out[1536:1664, :], q01[:])
                nc.sync.dma_start(y_out[1664:1792, :], kt2[:])
                return

            # ---- wo projection -> fp16 partial -> AllReduce ----
            wo0 = wob.tile([128, DM], F16, tag="wo0")
            nc.sync.dma_start(wo0[:], wo_in[l, 0:128, :])
            wo1 = wob.tile([128, DM], F16, tag="wo1")
            nc.sync.dma_start(wo1[:], wo_in[l, 128:256, :])
            cc_in = ccp.tile([DM, S], F16, tag="cc_in")
            cc_out = ccp.tile([DM, S], F16, tag="cc_out", name="cc_out",
                              addr_space="Shared")
            ctx_wo = ExitStack()
            ps_wo = ctx_wo.enter_context(tc.tile_pool(name="ps_wo", bufs=2, space="PSUM"))
            for grp in range(8):
                stg = stage.tile([128, 2048], F16, tag="wostg")
                for i in range(2):
                    dmm = grp * 2 + i
                    dsl = slice(dmm * 128, (dmm + 1) * 128)
                    wops = [ps_wo.tile([128, 512], F32, tag=f"wops{st}", name=f"wops{st}")
                            for st in range(2)]
                    for st in range(2):
                        nc.tensor.matmul(wops[st][:], wo0[:, dsl],
                                         stk0[:, st * 512:(st + 1) * 512],
                                         start=True, stop=False)
                    for st in range(2):
                        nc.tensor.matmul(wops[st][:], wo1[:, dsl],
                                         stk1[:, st * 512:(st + 1) * 512],
                                         start=False, stop=True)
                    for st in range(2):
                        osl = slice(i * 1024 + st * 512, i * 1024 + st * 512 + 512)
                        if st == 0:
                            nc.vector.tensor_copy(stg[:, osl], wops[st][:])
                        else:
                            nc.scalar.copy(stg[:, osl], wops[st][:])
                nc.scalar.dma_start(
                    bass.AP(tensor=cc_in.tensor,
                            offset=cc_in.offset + grp * 2 * 128 * S,
                            ap=[[S, 128], [128 * S, 2], [1, S]]),
                    stg[:])
            ctx_wo.close()
            if not skip_cc:
                nc.gpsimd.collective_compute(
                    "AllReduce", AluOpType.add,
                    replica_groups=[list(range(NCORES))],
                    ins=[cc_in[:].opt()], outs=[cc_out[:].opt()])
                residual_and_norm(cc_out)
            else:
                residual_and_norm(cc_in)

        def mlp_block(l, last):
            if not last:
                cc_in = ccp.tile([DM, S], F16, tag="cc_in")
                cc_out = ccp.tile([DM, S], F16, tag="cc_out", name="cc_out",
                                  addr_space="Shared")
            ctx_mlp = ExitStack()
            ps_mlp = ctx_mlp.enter_context(tc.tile_pool(name="ps_mlp", bufs=1, space="PSUM"))
            prod = {}
            for fmh in range(2):
                fms = [4 * fmh + j for j in range(4)]
                for phase, w_in in (("g", wg_in), ("u", wu_in)):
                    ps = {}
                    for fm in fms:
                        for st in range(2):
                            ps[(fm, st)] = ps_mlp.tile([128, 512], F32,
                                                       tag=f"m{fm % 4}_{st}",
                                                       name=f"m{phase}{fm}_{st}")
                    for g in range(8):
                        wt = wbig.tile([128, 1024], F16, tag="wgu", name="wgu")
                        nc.sync.dma_start(
                            wt[:],
                            bass.AP(tensor=w_in.tensor,
                                    offset=(w_in.offset + l * DM * FFS + g * 2 * 128 * FFS
                                            + fmh * 512),
                                    ap=[[FFS, 128], [128 * FFS, 2], [1, 512]]))
                        for i in range(2):
                            k = g * 2 + i
                            for fm in fms:
                                wsl = wt[:, i * 512 + (fm % 4) * 128:
                                         i * 512 + (fm % 4 + 1) * 128]
                                for st in range(2):
                                    nc.tensor.matmul(ps[(fm, st)][:], wsl,
                                                     XBN(k)[:, st * 512:(st + 1) * 512],
                                                     start=(k == 0), stop=(k == KT - 1))
                    if phase == "g":
                        sil = {}
                        for fm in fms:
                            for st in range(2):
                                t = actp.tile([128, 512], F16, tag=f"sil{fm % 4}_{st}",
                                              name=f"sil{fm}_{st}")
                                nc.scalar.activation(t[:], ps[(fm, st)][:], AF.Silu)
                                sil[(fm, st)] = t
                    else:
                        for fm in fms:
                            for st in range(2):
                                t = actp.tile([128, 512], F16, tag=f"prod{fm}_{st}",
                                              name=f"prod{fm}_{st}")
                                nc.vector.tensor_tensor(t[:], sil[(fm, st)][:],
                                                        ps[(fm, st)][:], AluOpType.mult)
                                prod[(fm, st)] = t
            for dmg in range(4):
                dps = {}
                for d in range(4):
                    for st in range(2):
                        dps[(d, st)] = ps_mlp.tile([128, 512], F32, tag=f"m{d}_{st}",
                                                   name=f"md{d}_{st}")
                for gg in range(2):
                    wdt = wbig.tile([128, 2048], F16, tag="wdt", name="wdt")
                    nc.sync.dma_start(
                        wdt[:],
                        bass.AP(tensor=wd_in.tensor,
                                offset=(wd_in.offset + l * FFS * DM + gg * 4 * 128 * DM
                                        + dmg * 512),
                                ap=[[DM, 128], [128 * DM, 4], [1, 512]]))
                    for i2 in range(4):
                        fk = gg * 4 + i2
                        for d in range(4):
                            wsl = wdt[:, i2 * 512 + d * 128:i2 * 512 + (d + 1) * 128]
                            for st in range(2):
                                nc.tensor.matmul(dps[(d, st)][:], wsl, prod[(fk, st)][:],
                                                 start=(fk == 0), stop=(fk == FMT - 1))
                for dp in range(2):
                    stg = stage.tile([128, 2048], F16, tag="dstg")
                    for i in range(2):
                        d = dp * 2 + i
                        kk = dmg * 4 + d
                        for st in range(2):
                            osl = slice(i * 1024 + st * 512, i * 1024 + st * 512 + 512)
                            if last:
                                nc.vector.scalar_tensor_tensor(
                                    stg[:, osl], XT(kk)[:, st * 512:(st + 1) * 512],
                                    1.0 / NCORES, dps[(d, st)][:],
                                    AluOpType.mult, AluOpType.add)
                            elif (2 * i + st) % 2 == 0:
                                nc.vector.tensor_copy(stg[:, osl], dps[(d, st)][:])
                            else:
                                nc.scalar.copy(stg[:, osl], dps[(d, st)][:])
                    dst = y_out if last else cc_in
                    nc.scalar.dma_start(
                        bass.AP(tensor=dst.tensor,
                                offset=dst.offset + (dmg * 4 + dp * 2) * 128 * S,
                                ap=[[S, 128], [128 * S, 2], [1, S]]),
                        stg[:])
            ctx_mlp.close()
            if not last:
                if not skip_cc:
                    nc.gpsimd.collective_compute(
                        "AllReduce", AluOpType.add,
                        replica_groups=[list(range(NCORES))],
                        ins=[cc_in[:].opt()], outs=[cc_out[:].opt()])
                    residual_and_norm(cc_out)
                else:
                    residual_and_norm(cc_in)

        def dump_x():
            for k in range(KT):
                nc.sync.dma_start(y_out[k * 128:(k + 1) * 128, :], XT(k))

        for _ in range(reps):
            load_x()
            for l in range(L):
                attn_block(l)
                if debug_stage == f"stk{l}":
                    break
                if debug_stage == f"attn{l}":
                    dump_x()
                    break
                mlp_block(l, last=(l == L - 1 and debug_stage is None))
                if debug_stage == f"mlp{l}":
                    dump_x()
                    break

    nc.compile()
    return nc


def make_inputs(input_ids, embed, wq, wk, wv, wo, wgate, wup, wdown, ln1, ln2):
    """host-side prep: embedding gather, layer0-norm, shard + fold gains."""
    f32 = np.float32
    f16 = np.float16
    x = np.asarray(embed, f32)[np.asarray(input_ids)[0]]      # (S, DM)
    xt = np.ascontiguousarray(x.T)                            # (DM, S)
    r0 = 1.0 / np.sqrt(np.mean(xt * xt, axis=0) + EPS)        # (S,)
    xbn0 = (xt * r0[None, :]).astype(f16)

    inv_freq = 1.0 / (THETA ** (np.arange(0, D, 2, dtype=f32) / D))
    freqs = np.arange(S, dtype=f32)[:, None] * inv_freq[None, :]    # (S, 32)
    emb = np.concatenate([freqs, freqs], axis=1)                    # (S, D)
    cosT = np.cos(emb).T.astype(f32)                                # (D, S)
    sinT = np.sin(emb).T.astype(f32)
    sinT_signed = sinT.copy()
    sinT_signed[: D // 2] *= -1.0
    cos_rep = np.concatenate([cosT, cosT], axis=0).astype(f16)      # (128, S)
    sin_rep = np.concatenate([sinT_signed, sinT_signed], axis=0).astype(f16)

    step = np.zeros((2, 128, 256), f16)
    jj = np.arange(128)[:, None]
    ii = np.arange(256)[None, :]
    for o in range(2):
        step[o] = (128 * o + jj > ii).astype(f16)
    negbigI = (NEGBIG * np.eye(128, dtype=f32)).astype(f16)
    ident = np.concatenate([np.eye(64), np.eye(64)], axis=0).astype(f16)

    scale_q = 1.0 / math.sqrt(D)
    in_maps = []
    for c in range(NCORES):
        wqkv = np.empty((L, DM, 384), f32)
        wo_c = np.empty((L, QC, DM), f32)
        wg_c = np.empty((L, DM, FFS), f32)
        wu_c = np.empty((L, DM, FFS), f32)
        wd_c = np.empty((L, FFS, DM), f32)
        for l in range(L):
            g1 = np.asarray(ln1[l], f32)[:, None]
            g2 = np.asarray(ln2[l], f32)[:, None]
            wqkv[l, :, :QC] = np.asarray(wq[l], f32)[:, c * QC:(c + 1) * QC] * g1 * scale_q
            wqkv[l, :, QC:QC + D] = np.asarray(wk[l], f32)[:, c * D:(c + 1) * D] * g1
            wqkv[l, :, QC + D:] = np.asarray(wv[l], f32)[:, c * D:(c + 1) * D] * g1
            wo_c[l] = np.asarray(wo[l], f32)[c * QC:(c + 1) * QC, :]
            wg_c[l] = np.asarray(wgate[l], f32)[:, c * FFS:(c + 1) * FFS] * g2
            wu_c[l] = np.asarray(wup[l], f32)[:, c * FFS:(c + 1) * FFS] * g2
            wd_c[l] = np.asarray(wdown[l], f32)[c * FFS:(c + 1) * FFS, :]
        in_maps.append({
            "xt16": xt.astype(f16), "xbn0": xbn0, "wqkv": wqkv.astype(f16),
            "wo": wo_c.astype(f16), "wg": wg_c.astype(f16), "wu": wu_c.astype(f16),
            "wd": wd_c.astype(f16), "cosr": cos_rep, "sinr": sin_rep,
            "step": step, "negbigI": negbigI, "ident": ident,
        })
    return in_maps


_NC_CACHE = {}


def kernel(**inputs) -> np.ndarray:
    if 1 not in _NC_CACHE:
        _NC_CACHE[1] = build(reps=1)
    nc = _NC_CACHE[1]
    in_maps = make_inputs(**inputs)
    res = run_bass_kernel_spmd(nc, in_maps, list(range(NCORES)))
    y = np.zeros((DM, S), np.float64)
    for c in range(NCORES):
        y += res.results[c]["y"].astype(np.float64)
    return np.ascontiguousarray(y.T.astype(np.float32)).reshape(B, S, DM)



# revision 9
# speedup vs baseline: 1.0497x; 1.0497x over previous
"""Llama trunk (2 layers, before final norm) on 8 trn2 cores.

Sharding: Megatron tensor-parallel over 8 cores.
  - attention: 4 q-heads + 1 kv-head per core (GQA group stays local)
  - MLP: 1024 of 8192 intermediate dims per core
  - residual stream xt kept in fp16, transposed [DM(part), S(free)]

v2: token-half software pipeline. All per-token phases (qkv, rope, wo,
norm, MLP) are processed in two 512-column halves T0/T1, and the three
on-device AllReduces are split into six half-payload AllReduces, each
overlapped with the other half's compute (collectives run on dedicated
SDMA/CCE silicon, so the PE keeps streaming through them). Attention is
causal, so T0 queries only need T0 keys and the attention block also
pipelines by half. Other changes vs v1:
  - softmax 1/sum is broadcast to the 64-row head blocks with two K=1
    outer-product matmuls into PSUM (ones_lo/ones_hi stationaries)
    instead of a DRAM round-trip.
  - qkv is two passes (q-pass: 2 psums, kv-pass: 1) so attention fits
    the 8 PSUM banks together with wo / norm psums; weights stay in
    SBUF across both passes and both halves.
  - odd-parity softmax key-sums go to one packed PSUM bank (partition
    0 / 32 via tile_position) instead of two.
"""
import math
from contextlib import ExitStack

import numpy as np

import concourse.bass as bass
import concourse.tile as tile
from concourse import bacc, mybir
from concourse.alu_op_type import AluOpType
from concourse.bass_utils import run_bass_kernel_spmd

L, H, KVH, D = 2, 32, 8, 64
DM, FF = 2048, 8192
B, S = 1, 1024
EPS, THETA = 1e-5, 10000.0
NCORES = 8
QH = H // NCORES            # 4 q heads per core
QC = QH * D                 # 256 q cols per core
FFS = FF // NCORES          # 1024 ff dims per core
KT = DM // 128              # 16 contraction tiles over DM
FMT = FFS // 128            # 8 f tiles
HS = S // 2                 # 512 token half
NEGBIG = -30000.0

F32 = mybir.dt.float32
F16 = mybir.dt.float16
AF = mybir.ActivationFunctionType


def build(reps=1, debug_stage=None, skip_cc=False):
    nc = bacc.Bacc(None, target_bir_lowering=False, debug=False, num_devices=NCORES)
    xt_in = nc.dram_tensor("xt16", [DM, S], F16, kind="ExternalInput").ap()
    xbn0_in = nc.dram_tensor("xbn0", [DM, S], F16, kind="ExternalInput").ap()
    wqkv_in = nc.dram_tensor("wqkv", [L, DM, 384], F16, kind="ExternalInput").ap()
    wo_in = nc.dram_tensor("wo", [L, QC, DM], F16, kind="ExternalInput").ap()
    wg_in = nc.dram_tensor("wg", [L, DM, FFS], F16, kind="ExternalInput").ap()
    wu_in = nc.dram_tensor("wu", [L, DM, FFS], F16, kind="ExternalInput").ap()
    wd_in = nc.dram_tensor("wd", [L, FFS, DM], F16, kind="ExternalInput").ap()
    cos_in = nc.dram_tensor("cosr", [128, S], F16, kind="ExternalInput").ap()
    sin_in = nc.dram_tensor("sinr", [128, S], F16, kind="ExternalInput").ap()
    step_in = nc.dram_tensor("step", [2, 128, 256], F16, kind="ExternalInput").ap()
    nbi_in = nc.dram_tensor("negbigI", [128, 128], F16, kind="ExternalInput").ap()
    id_in = nc.dram_tensor("ident", [128, 64], F16, kind="ExternalInput").ap()
    y_out = nc.dram_tensor("y", [DM, S], F16, kind="ExternalOutput").ap()

    with tile.TileContext(nc) as tc, ExitStack() as ctx, \
            nc.allow_low_precision(reason="deliberate fp16 pipeline, tol 2e-2"):
        const = ctx.enter_context(tc.tile_pool(name="const", bufs=1))
        xtp = ctx.enter_context(tc.tile_pool(name="xtp", bufs=1))
        wqp = ctx.enter_context(tc.tile_pool(name="wqp", bufs=1))
        wob = ctx.enter_context(tc.tile_pool(name="wob", bufs=1))
        wbig = ctx.enter_context(tc.tile_pool(name="wbig", bufs=2))
        sq = ctx.enter_context(tc.tile_pool(name="sq", bufs=2))
        rp = ctx.enter_context(tc.tile_pool(name="rp", bufs=2))
        attn_sb = ctx.enter_context(tc.tile_pool(name="attn_sb", bufs=1))
        ropet = ctx.enter_context(tc.tile_pool(name="ropet", bufs=2))
        vap = ctx.enter_context(tc.tile_pool(name="vap", bufs=1))
        expp = ctx.enter_context(tc.tile_pool(name="expp", bufs=2))
        stkp = ctx.enter_context(tc.tile_pool(name="stkp", bufs=1))
        stage = ctx.enter_context(tc.tile_pool(name="stage", bufs=2))
        arp = ctx.enter_context(tc.tile_pool(name="arp", bufs=2))
        actp = ctx.enter_context(tc.tile_pool(name="actp", bufs=1))

        dram = ctx.enter_context(tc.tile_pool(name="dram", bufs=2, space="DRAM"))
        # persistent psum: norm sumsq / odd softmax sums (1 bank) + row
        # broadcasts (1 bank)
        pnorm = ctx.enter_context(tc.tile_pool(name="pnorm", bufs=1, space="PSUM"))

        # ---- persistent constants ----
        onesb = const.tile([128, 1], F16)
        nc.vector.memset(onesb[:], 1.0)
        onesr = const.tile([1, 128], F16)
        nc.vector.memset(onesr[:], 1.0)
        ones_lo = const.tile([1, 128], F16)
        nc.vector.memset(ones_lo[0:1, 0:64], 1.0)
        nc.vector.memset(ones_lo[0:1, 64:128], 0.0)
        ones_hi = const.tile([1, 128], F16)
        nc.vector.memset(ones_hi[0:1, 0:64], 0.0)
        nc.vector.memset(ones_hi[0:1, 64:128], 1.0)
        cosr = const.tile([128, S], F16)
        nc.sync.dma_start(cosr[:], cos_in[:])
        sinr = const.tile([128, S], F16)
        nc.sync.dma_start(sinr[:], sin_in[:])
        step0 = const.tile([128, 256], F16)
        nc.sync.dma_start(step0[:], step_in[0, :, :])
        step1 = const.tile([128, 256], F16)
        nc.sync.dma_start(step1[:], step_in[1, :, :])
        nbi = const.tile([128, 128], F16)
        nc.sync.dma_start(nbi[:], nbi_in[:])
        ident = const.tile([128, 64], F16)
        nc.sync.dma_start(ident[:], id_in[:])
        epsb = const.tile([1, 1], F32)
        nc.vector.memset(epsb[:], EPS)

        # residual stream + normalized copy, fp16, resident (big tiles)
        xtb = xtp.tile([128, KT * S], F16, tag="xtb", name="xtb")
        xbnb = xtp.tile([128, KT * S], F16, tag="xbnb", name="xbnb")

        def XT(k, st=None):
            if st is None:
                return xtb[:, k * S:(k + 1) * S]
            return xtb[:, k * S + st * HS:k * S + (st + 1) * HS]

        def XBN(k, st=None):
            if st is None:
                return xbnb[:, k * S:(k + 1) * S]
            return xbnb[:, k * S + st * HS:k * S + (st + 1) * HS]

        xtb3 = xtb.rearrange("p (k s) -> p k s", k=KT)
        xbnb3 = xbnb.rearrange("p (k s) -> p k s", k=KT)

        def load_x():
            # xbn0 halves first: layer-0 qkv(T0) only needs xbn0(T0)
            for st in range(2):
                nc.sync.dma_start(
                    xbnb3[:, :, st * HS:(st + 1) * HS],
                    bass.AP(tensor=xbn0_in.tensor, offset=xbn0_in.offset + st * HS,
                            ap=[[S, 128], [128 * S, KT], [1, HS]]))
            for st in range(2):
                nc.sync.dma_start(
                    xtb3[:, :, st * HS:(st + 1) * HS],
                    bass.AP(tensor=xt_in.tensor, offset=xt_in.offset + st * HS,
                            ap=[[S, 128], [128 * S, KT], [1, HS]]))

        # ---------- per layer-token-half pieces ----------

        def residual_and_norm_half(pa, cc_out, st):
            """xt(st) += AR result; sumsq stats; r; xbn(st)."""
            stsl = slice(st * HS, (st + 1) * HS)
            ssum = pnorm.tile([1, HS], F32, tag="ssum", name="ssum")
            for q in range(4):
                ar_w = arp.tile([128, 4 * HS], F16, tag="ar")
                nc.sync.dma_start(
                    ar_w[:],
                    bass.AP(tensor=cc_out.tensor, offset=cc_out.offset + q * 4 * 128 * HS,
                            ap=[[HS, 128], [128 * HS, 4], [1, HS]]))
                for i in range(4):
                    k = q * 4 + i
                    nc.vector.tensor_add(XT(k, st), XT(k, st), ar_w[:, i * HS:(i + 1) * HS])
                    xsq = sq.tile([128, HS], F16, tag="xsq")
                    nc.vector.tensor_tensor(xsq[:], XT(k, st), XT(k, st), AluOpType.mult)
                    nc.tensor.matmul(ssum[:], onesb[:], xsq[:],
                                     start=(k == 0), stop=(k == KT - 1))
            rs = rp.tile([1, HS], F32, tag="rs")
            nc.scalar.activation(rs[:], ssum[:], AF.Sqrt, bias=epsb[:], scale=1.0 / DM)
            rrf = rp.tile([1, HS], F32, tag="rrf")
            nc.vector.reciprocal(rrf[:], rs[:])
            rr16 = rp.tile([1, HS], F16, tag="rr16")
            nc.vector.tensor_copy(rr16[:], rrf[:])
            rb_ps = pnorm.tile([128, HS], F32, tag="rbp", name="rb_ps")
            nc.tensor.matmul(rb_ps[:], onesr[:], rr16[:], start=True, stop=True)
            rb = rp.tile([128, HS], F16, tag="rb")
            nc.vector.tensor_copy(rb[:], rb_ps[:])
            for k in range(KT):
                nc.vector.tensor_tensor(XBN(k, st), XT(k, st), rb[:], AluOpType.mult)

        def qkv_half(pa, wqa, q01, q23, kv2, st):
            stsl = slice(st * HS, (st + 1) * HS)
            # q-pass: two psums accumulate over all 16 k-tiles
            pq0 = pa.tile([128, HS], F32, tag="pq0", name="pq0")
            pq1 = pa.tile([128, HS], F32, tag="pq1", name="pq1")
            for k in range(KT):
                w0 = k * 384
                st_, sp_ = (k == 0), (k == KT - 1)
                nc.tensor.matmul(pq0[:], wqa[:, w0:w0 + 128], XBN(k, st),
                                 start=st_, stop=sp_)
                nc.tensor.matmul(pq1[:], wqa[:, w0 + 128:w0 + 256], XBN(k, st),
                                 start=st_, stop=sp_)
            nc.vector.tensor_copy(q01[:, stsl], pq0[:])
            nc.vector.tensor_copy(q23[:, stsl], pq1[:])
            # kv-pass: one psum
            pkv = pa.tile([128, HS], F32, tag="sps0", name="pkv")
            for k in range(KT):
                w0 = k * 384
                nc.tensor.matmul(pkv[:], wqa[:, w0 + 256:w0 + 384], XBN(k, st),
                                 start=(k == 0), stop=(k == KT - 1))
            # k rows first so rope-k starts while v evacuates
            nc.scalar.copy(kv2[0:64, stsl], pkv[0:64, :])
            nc.scalar.copy(kv2[64:128, stsl], pkv[64:128, :])

        def v_transpose_half(pa, kv2, va, st):
            for sj in range(4 * st, 4 * st + 4):
                tp = pa.tile([128, 64], F16, tag=f"aps{sj % 2}", name="tp")
                nc.tensor.transpose(tp[:], kv2[64:128, sj * 128:(sj + 1) * 128],
                                    ident[64:128, :])
                v = vap.tile([128, 65], F16, tag=f"va{sj}", name=f"va{sj}")
                nc.vector.tensor_copy(v[:, 0:64], tp[:])
                nc.vector.memset(v[:, 64:65], 1.0)
                va.append(v)

        def rope_half(q01, q23, kv2, kt2, st):
            stsl = slice(st * HS, (st + 1) * HS)

            def rope(t, nrows, out, outrows):
                rot = ropet.tile([128, HS], F16, tag="rot")
                for h0 in range(0, nrows, 64):
                    nc.scalar.dma_start(rot[h0:h0 + 32, :], t[h0 + 32:h0 + 64, stsl])
                    nc.scalar.dma_start(rot[h0 + 32:h0 + 64, :], t[h0:h0 + 32, stsl])
                t1 = ropet.tile([128, HS], F16, tag="t1")
                nc.vector.tensor_tensor(t1[0:nrows, :], t[0:nrows, stsl],
                                        cosr[0:nrows, stsl], AluOpType.mult)
                t2 = ropet.tile([128, HS], F16, tag="t2")
                nc.vector.tensor_tensor(t2[0:nrows, :], rot[0:nrows, :],
                                        sinr[0:nrows, stsl], AluOpType.mult)
                nc.vector.tensor_add(out[outrows, stsl], t1[0:nrows, :], t2[0:nrows, :])
            rope(q01, 128, q01, slice(0, 128))
            rope(q23, 128, q23, slice(0, 128))
            rope(kv2, 64, kt2, slice(0, 64))
            nc.scalar.dma_start(kt2[64:128, stsl], kt2[0:64, stsl])

        def scav_half(pa, q01, q23, kt2, va, stk0, stk1, sinv_sb, st):
            """scores+exp+av for query blocks of half st (it in 2st..2st+1).

            it-outer so both parities of a query block finish together and
            the block can be normalized while the next block's scores run.
            """
            jp_ctr = 0
            for it in range(2 * st, 2 * st + 2):
                for parity in range(2):
                    rows = slice(64 * parity, 64 * parity + 64)
                    odd = parity == 1
                    isl = slice(it * 256, (it + 1) * 256)
                    aps = [pa.tile([128, 512], F32, tag=f"aps{m}", name=f"aps{m}")
                           for m in range(2)]
                    for jp in range(it + 1):
                        diag = jp == it
                        tg = ("pq0", "pq1") if jp_ctr % 2 else ("sps0", "sps1")
                        jp_ctr += 1
                        sps = [pa.tile([128, 512], F32, tag=tg[m], name=f"sps{m}")
                               for m in range(2)]
                        for half in range(2):
                            j = 2 * jp + half
                            ssl = slice(half * 256, half * 256 + 256)
                            if diag:
                                for m in range(2):
                                    nc.tensor.matmul(sps[m][:, ssl], nbi[:],
                                                     (step0, step1)[half][:],
                                                     start=True, stop=False)
                            for m, qt in enumerate((q01, q23)):
                                nc.tensor.matmul(sps[m][:, ssl],
                                                 kt2[rows, j * 128:(j + 1) * 128],
                                                 qt[rows, isl],
                                                 start=not diag, stop=True)
                        es = []
                        for m in range(2):
                            e = expp.tile([128, 512], F16, tag=f"e{m}", name=f"e{m}")
                            nc.scalar.activation(e[:], sps[m][:], AF.Exp)
                            es.append(e)
                        first, last = (jp == 0), (jp == it)
                        for half in range(2):
                            esl = slice(half * 256, half * 256 + 256)
                            vsl = slice(0, 64) if odd else slice(0, 65)
                            orows = slice(64, 128) if odd else slice(0, 65)
                            for m in range(2):
                                # odd parity: av (rows 64:128) and key-sums
                                # (row 0) share a bank as two accumulation
                                # groups on disjoint partitions — PSUM
                                # pending-zero tracking is per-partition.
                                nc.tensor.matmul(aps[m][orows, 0:256],
                                                 va[2 * jp + half][:, vsl], es[m][:, esl],
                                                 start=(first and half == 0),
                                                 stop=(last and half == 1))
                            if odd:
                                for m in range(2):
                                    nc.tensor.matmul(
                                        aps[m][0:1, 0:256],
                                        onesb[:], es[m][:, esl],
                                        start=(first and half == 0),
                                        stop=(last and half == 1))
                    arows = slice(64, 128) if odd else slice(0, 64)
                    heads = (parity, parity + 2)
                    for m in range(2):
                        h = heads[m]
                        srow = (aps[m][0:1, 0:256] if odd
                                else aps[m][64:65, 0:256])
                        nc.vector.reciprocal(
                            sinv_sb[0:1, h * S + it * 256:h * S + it * 256 + 256], srow)
                        nc.vector.tensor_copy((stk0, stk1)[m][arows, isl],
                                              aps[m][arows, 0:256])
                # normalize this 256-col query block while the next block's
                # scores run: 1/sums broadcast to the two 64-row head blocks
                # via K=1 outer products, multiplied straight out of PSUM.
                for t, h0, h1 in ((stk0, 0, 1), (stk1, 2, 3)):
                    rb_ps = pnorm.tile([128, 256], F32, tag="rbp", name="sinv_ps")
                    nc.tensor.matmul(rb_ps[:], ones_lo[:],
                                     sinv_sb[0:1, h0 * S + it * 256:h0 * S + it * 256 + 256],
                                     start=True, stop=False)
                    nc.tensor.matmul(rb_ps[:], ones_hi[:],
                                     sinv_sb[0:1, h1 * S + it * 256:h1 * S + it * 256 + 256],
                                     start=False, stop=True)
                    nc.vector.tensor_tensor(t[:, isl], t[:, isl], rb_ps[:],
                                            AluOpType.mult)

        def wo_half(pa, wo0, wo1, stk0, stk1, cc_in, st):
            stsl = slice(st * HS, (st + 1) * HS)
            wo_tags = ("pq0", "pq1", "sps0", "sps1")
            for grp in range(8):
                stg = stage.tile([128, 1024], F16, tag="wostg")
                for i in range(2):
                    dmm = grp * 2 + i
                    dsl = slice(dmm * 128, (dmm + 1) * 128)
                    wops = pa.tile([128, HS], F32, tag=wo_tags[dmm % 4], name="wops")
                    nc.tensor.matmul(wops[:], wo0[:, dsl], stk0[:, stsl],
                                     start=True, stop=False)
                    nc.tensor.matmul(wops[:], wo1[:, dsl], stk1[:, stsl],
                                     start=False, stop=True)
                    osl = slice(i * HS, (i + 1) * HS)
                    if i == 0:
                        nc.vector.tensor_copy(stg[:, osl], wops[:])
                    else:
                        nc.scalar.copy(stg[:, osl], wops[:])
                nc.scalar.dma_start(
                    bass.AP(tensor=cc_in.tensor,
                            offset=cc_in.offset + grp * 2 * 128 * HS,
                            ap=[[HS, 128], [128 * HS, 2], [1, HS]]),
                    stg[:])

        def gu_half(pm, l, prod, st):
            for fmh in range(2):
                fms = [4 * fmh + j for j in range(4)]
                for phase, w_in in (("g", wg_in), ("u", wu_in)):
                    ps = {}
                    for fm in fms:
                        ps[fm] = pm.tile([128, HS], F32, tag=f"m{fm % 4}",
                                         name=f"m{phase}{fm}")
                    for g in range(8):
                        wt = wbig.tile([128, 1024], F16, tag="wgu", name="wgu")
                        nc.sync.dma_start(
                            wt[:],
                            bass.AP(tensor=w_in.tensor,
                                    offset=(w_in.offset + l * DM * FFS + g * 2 * 128 * FFS
                                            + fmh * 512),
                                    ap=[[FFS, 128], [128 * FFS, 2], [1, 512]]))
                        for i in range(2):
                            k = g * 2 + i
                            for fm in fms:
                                wsl = wt[:, i * 512 + (fm % 4) * 128:
                                         i * 512 + (fm % 4 + 1) * 128]
                                nc.tensor.matmul(ps[fm][:], wsl, XBN(k, st),
                                                 start=(k == 0), stop=(k == KT - 1))
                    if phase == "g":
                        sil = {}
                        for fm in fms:
                            t = actp.tile([128, HS], F16, tag=f"sil{fm % 4}",
                                          name=f"sil{fm}")
                            nc.scalar.activation(t[:], ps[fm][:], AF.Silu)
                            sil[fm] = t
                    else:
                        for fm in fms:
                            t = actp.tile([128, HS], F16, tag=f"prod{fm}",
                                          name=f"prod{fm}")
                            nc.vector.tensor_tensor(t[:], sil[fm][:], ps[fm][:],
                                                    AluOpType.mult)
                            prod[fm] = t

        def down_half(pm, l, prod, cc_in, st, last):
            stsl = slice(st * HS, (st + 1) * HS)
            for dmg in range(4):
                dps = {}
                for d in range(4):
                    dps[d] = pm.tile([128, HS], F32, tag=f"m{d}", name=f"md{d}")
                for gg in range(2):
                    wdt = wbig.tile([128, 2048], F16, tag="wdt", name="wdt")
                    nc.sync.dma_start(
                        wdt[:],
                        bass.AP(tensor=wd_in.tensor,
                                offset=(wd_in.offset + l * FFS * DM + gg * 4 * 128 * DM
                                        + dmg * 512),
                                ap=[[DM, 128], [128 * DM, 4], [1, 512]]))
                    for i2 in range(4):
                        fk = gg * 4 + i2
                        for d in range(4):
                            wsl = wdt[:, i2 * 512 + d * 128:i2 * 512 + (d + 1) * 128]
                            nc.tensor.matmul(dps[d][:], wsl, prod[fk][:],
                                             start=(fk == 0), stop=(fk == FMT - 1))
                for dp in range(2):
                    stg = stage.tile([128, 1024], F16, tag="dstg")
                    for i in range(2):
                        d = dp * 2 + i
                        kk = dmg * 4 + d
                        osl = slice(i * HS, (i + 1) * HS)
                        if last:
                            nc.vector.scalar_tensor_tensor(
                                stg[:, osl], XT(kk, st), 1.0 / NCORES, dps[d][:],
                                AluOpType.mult, AluOpType.add)
                        elif i == 0:
                            nc.vector.tensor_copy(stg[:, osl], dps[d][:])
                        else:
                            nc.scalar.copy(stg[:, osl], dps[d][:])
                    dst = y_out if last else cc_in
                    if last:
                        dstap = bass.AP(
                            tensor=dst.tensor,
                            offset=dst.offset + (dmg * 4 + dp * 2) * 128 * S + st * HS,
                            ap=[[S, 128], [128 * S, 2], [1, HS]])
                    else:
                        dstap = bass.AP(
                            tensor=dst.tensor,
                            offset=dst.offset + (dmg * 4 + dp * 2) * 128 * HS,
                            ap=[[HS, 128], [128 * HS, 2], [1, HS]])
                    nc.scalar.dma_start(dstap, stg[:])

        def allreduce(cc_in, cc_out):
            nc.gpsimd.collective_compute(
                "AllReduce", AluOpType.add,
                replica_groups=[list(range(NCORES))],
                ins=[cc_in[:].opt()], outs=[cc_out[:].opt()])

        def cc_pair(tagbase, st):
            cc_in = dram.tile([DM, HS], F16, tag=f"{tagbase}i{st}", name="cc_in")
            if skip_cc:
                return cc_in, cc_in
            cc_out = dram.tile([DM, HS], F16, tag=f"{tagbase}o{st}", name="cc_out",
                               addr_space="Shared")
            return cc_in, cc_out

        # ---------- main program ----------
        for _ in range(reps):
            load_x()
            # mlp-norm carried across the layer boundary
            pend_mlp_cc = [None, None]
            for l in range(L):
                ctx_a = ExitStack()
                pa = ctx_a.enter_context(tc.tile_pool(name="pa", bufs=1, space="PSUM"))
                wqa = wqp.tile([128, KT * 384], F16, tag="wqa", name="wqa")
                for g in range(4):
                    nc.sync.dma_start(
                        wqa[:, g * 4 * 384:(g + 1) * 4 * 384],
                        bass.AP(tensor=wqkv_in.tensor,
                                offset=wqkv_in.offset + l * DM * 384 + g * 4 * 128 * 384,
                                ap=[[384, 128], [128 * 384, 4], [1, 384]]))
                wo0 = wob.tile([128, DM], F16, tag="wo0")
                nc.sync.dma_start(wo0[:], wo_in[l, 0:128, :])
                wo1 = wob.tile([128, DM], F16, tag="wo1")
                nc.sync.dma_start(wo1[:], wo_in[l, 128:256, :])

                q01 = attn_sb.tile([128, S], F16, tag="q01")
                q23 = attn_sb.tile([128, S], F16, tag="q23")
                kv2 = attn_sb.tile([128, S], F16, tag="kv2")
                kt2 = attn_sb.tile([128, S], F16, tag="kt2")
                stk0 = stkp.tile([128, S], F16, tag="stk0")
                stk1 = stkp.tile([128, S], F16, tag="stk1")
                sinv_sb = stkp.tile([1, 4 * S], F16, tag="sinv_sb")
                va = []
                cc_a = [cc_pair("a", st) for st in range(2)]

                for st in range(2):
                    if pend_mlp_cc[st] is not None:
                        residual_and_norm_half(pa, pend_mlp_cc[st], st)
                        pend_mlp_cc[st] = None
                    qkv_half(pa, wqa, q01, q23, kv2, st)
                    v_transpose_half(pa, kv2, va, st)
                    rope_half(q01, q23, kv2, kt2, st)
                    scav_half(pa, q01, q23, kt2, va, stk0, stk1, sinv_sb, st)
                    normalize_half(pa, stk0, stk1, sinv_sb, st)
                    wo_half(pa, wo0, wo1, stk0, stk1, cc_a[st][0], st)
                    if not skip_cc:
                        allreduce(cc_a[st][0], cc_a[st][1])

                if debug_stage == f"stk{l}":
                    for ti, t in enumerate((stk0, stk1)):
                        nc.sync.dma_start(y_out[ti * 128:(ti + 1) * 128, :], t[:])
                    ctx_a.close()
                    break
                ctx_a.close()

                last = (l == L - 1 and debug_stage is None)
                ctx_m = ExitStack()
                pm = ctx_m.enter_context(tc.tile_pool(name="pm", bufs=1, space="PSUM"))
                cc_m = [None if last else cc_pair("m", st) for st in range(2)]
                for st in range(2):
                    residual_and_norm_half(pm, cc_a[st][1], st)
                    prod = {}
                    gu_half(pm, l, prod, st)
                    down_half(pm, l, prod, None if last else cc_m[st][0], st, last)
                    if not last and not skip_cc:
                        allreduce(cc_m[st][0], cc_m[st][1])
                    if not last:
                        pend_mlp_cc[st] = cc_m[st][1]
                ctx_m.close()
                if debug_stage == f"attn{l}" or debug_stage == f"mlp{l}":
                    for k in range(KT):
                        nc.sync.dma_start(y_out[k * 128:(k + 1) * 128, :], XT(k))
                    break

    nc.compile()
    return nc


def make_inputs(input_ids, embed, wq, wk, wv, wo, wgate, wup, wdown, ln1, ln2):
    """host-side prep: embedding gather, layer0-norm, shard + fold gains."""
    f32 = np.float32
    f16 = np.float16
    x = np.asarray(embed, f32)[np.asarray(input_ids)[0]]      # (S, DM)
    xt = np.ascontiguousarray(x.T)                            # (DM, S)
    r0 = 1.0 / np.sqrt(np.mean(xt * xt, axis=0) + EPS)        # (S,)
    xbn0 = (xt * r0[None, :]).astype(f16)

    inv_freq = 1.0 / (THETA ** (np.arange(0, D, 2, dtype=f32) / D))
    freqs = np.arange(S, dtype=f32)[:, None] * inv_freq[None, :]    # (S, 32)
    emb = np.concatenate([freqs, freqs], axis=1)                    # (S, D)
    cosT = np.cos(emb).T.astype(f32)                                # (D, S)
    sinT = np.sin(emb).T.astype(f32)
    sinT_signed = sinT.copy()
    sinT_signed[: D // 2] *= -1.0
    cos_rep = np.concatenate([cosT, cosT], axis=0).astype(f16)      # (128, S)
    sin_rep = np.concatenate([sinT_signed, sinT_signed], axis=0).astype(f16)

    step = np.zeros((2, 128, 256), f16)
    jj = np.arange(128)[:, None]
    ii = np.arange(256)[None, :]
    for o in range(2):
        step[o] = (128 * o + jj > ii).astype(f16)
    negbigI = (NEGBIG * np.eye(128, dtype=f32)).astype(f16)
    ident = np.concatenate([np.eye(64), np.eye(64)], axis=0).astype(f16)

    scale_q = 1.0 / math.sqrt(D)
    in_maps = []
    for c in range(NCORES):
        wqkv = np.empty((L, DM, 384), f32)
        wo_c = np.empty((L, QC, DM), f32)
        wg_c = np.empty((L, DM, FFS), f32)
        wu_c = np.empty((L, DM, FFS), f32)
        wd_c = np.empty((L, FFS, DM), f32)
        for l in range(L):
            g1 = np.asarray(ln1[l], f32)[:, None]
            g2 = np.asarray(ln2[l], f32)[:, None]
            wqkv[l, :, :QC] = np.asarray(wq[l], f32)[:, c * QC:(c + 1) * QC] * g1 * scale_q
            wqkv[l, :, QC:QC + D] = np.asarray(wk[l], f32)[:, c * D:(c + 1) * D] * g1
            wqkv[l, :, QC + D:] = np.asarray(wv[l], f32)[:, c * D:(c + 1) * D] * g1
            wo_c[l] = np.asarray(wo[l], f32)[c * QC:(c + 1) * QC, :]
            wg_c[l] = np.asarray(wgate[l], f32)[:, c * FFS:(c + 1) * FFS] * g2
            wu_c[l] = np.asarray(wup[l], f32)[:, c * FFS:(c + 1) * FFS] * g2
            wd_c[l] = np.asarray(wdown[l], f32)[c * FFS:(c + 1) * FFS, :]
        in_maps.append({
            "xt16": xt.astype(f16), "xbn0": xbn0, "wqkv": wqkv.astype(f16),
            "wo": wo_c.astype(f16), "wg": wg_c.astype(f16), "wu": wu_c.astype(f16),
            "wd": wd_c.astype(f16), "cosr": cos_rep, "sinr": sin_rep,
            "step": step, "negbigI": negbigI, "ident": ident,
        })
    return in_maps


_NC_CACHE = {}


def kernel(**inputs) -> np.ndarray:
    if 1 not in _NC_CACHE:
        _NC_CACHE[1] = build(reps=1)
    nc = _NC_CACHE[1]
    in_maps = make_inputs(**inputs)
    res = run_bass_kernel_spmd(nc, in_maps, list(range(NCORES)))
    y = np.zeros((DM, S), np.float64)
    for c in range(NCORES):
        y += res.results[c]["y"].astype(np.float64)
    return np.ascontiguousarray(y.T.astype(np.float32)).reshape(B, S, DM)


# revision 33
# speedup vs baseline: 1.1164x; 1.0635x over previous
"""Llama trunk (2 layers, before final norm) on 8 trn2 cores.

Sharding: Megatron tensor-parallel over 8 cores.
  - attention: 4 q-heads + 1 kv-head per core (GQA group stays local)
  - MLP: 1024 of 8192 intermediate dims per core
  - residual stream xt kept in fp16, transposed [DM(part), S(free)]

v2: token-half software pipeline. All per-token phases (qkv, rope, wo,
norm, MLP) are processed in two 512-column halves T0/T1, and the three
on-device AllReduces are split into six half-payload AllReduces, each
overlapped with the other half's compute (collectives run on dedicated
SDMA/CCE silicon, so the PE keeps streaming through them). Attention is
causal, so T0 queries only need T0 keys and the attention block also
pipelines by half. Other changes vs v1:
  - softmax 1/sum is broadcast to the 64-row head blocks with two K=1
    outer-product matmuls into PSUM (ones_lo/ones_hi stationaries)
    instead of a DRAM round-trip.
  - qkv is two passes (q-pass: 2 psums, kv-pass: 1) so attention fits
    the 8 PSUM banks together with wo / norm psums; weights stay in
    SBUF across both passes and both halves.
  - odd-parity softmax key-sums go to one packed PSUM bank (partition
    0 / 32 via tile_position) instead of two.
"""
import math
from contextlib import ExitStack

import numpy as np

import concourse.bass as bass
import concourse.tile as tile
from concourse import bacc, mybir
from concourse.alu_op_type import AluOpType
from concourse.bass_utils import run_bass_kernel_spmd

L, H, KVH, D = 2, 32, 8, 64
DM, FF = 2048, 8192
B, S = 1, 1024
EPS, THETA = 1e-5, 10000.0
NCORES = 8
QH = H // NCORES            # 4 q heads per core
QC = QH * D                 # 256 q cols per core
FFS = FF // NCORES          # 1024 ff dims per core
KT = DM // 128              # 16 contraction tiles over DM
FMT = FFS // 128            # 8 f tiles
HS = S // 2                 # 512 token half
NEGBIG = -30000.0

F32 = mybir.dt.float32
F16 = mybir.dt.float16
AF = mybir.ActivationFunctionType


def build(reps=1, debug_stage=None, skip_cc=False):
    nc = bacc.Bacc(None, target_bir_lowering=False, debug=False, num_devices=NCORES)
    xt_in = nc.dram_tensor("xt16", [DM, S], F16, kind="ExternalInput").ap()
    xbn0_in = nc.dram_tensor("xbn0", [DM, S], F16, kind="ExternalInput").ap()
    wqkv_in = nc.dram_tensor("wqkv", [L, DM, 384], F16, kind="ExternalInput").ap()
    wo_in = nc.dram_tensor("wo", [L, QC, DM], F16, kind="ExternalInput").ap()
    wg_in = nc.dram_tensor("wg", [L, DM, FFS], F16, kind="ExternalInput").ap()
    wu_in = nc.dram_tensor("wu", [L, DM, FFS], F16, kind="ExternalInput").ap()
    wd_in = nc.dram_tensor("wd", [L, FFS, DM], F16, kind="ExternalInput").ap()
    cos_in = nc.dram_tensor("cosr", [128, S], F16, kind="ExternalInput").ap()
    sin_in = nc.dram_tensor("sinr", [128, S], F16, kind="ExternalInput").ap()
    cpack_in = nc.dram_tensor("cpack", [128, 704], F16, kind="ExternalInput").ap()
    y_out = nc.dram_tensor("y", [DM, S], F16, kind="ExternalOutput").ap()

    with tile.TileContext(nc) as tc, ExitStack() as ctx, \
            nc.allow_low_precision(reason="deliberate fp16 pipeline, tol 2e-2"):
        const = ctx.enter_context(tc.tile_pool(name="const", bufs=1))
        xtp = ctx.enter_context(tc.tile_pool(name="xtp", bufs=1))
        wqp = ctx.enter_context(tc.tile_pool(name="wqp", bufs=1))
        wob = ctx.enter_context(tc.tile_pool(name="wob", bufs=1))
        wbig = ctx.enter_context(tc.tile_pool(name="wbig", bufs=2))
        sq = ctx.enter_context(tc.tile_pool(name="sq", bufs=2))
        rp = ctx.enter_context(tc.tile_pool(name="rp", bufs=2))
        attn_sb = ctx.enter_context(tc.tile_pool(name="attn_sb", bufs=1))
        ropet = ctx.enter_context(tc.tile_pool(name="ropet", bufs=2))
        vap = ctx.enter_context(tc.tile_pool(name="vap", bufs=1))
        expp = ctx.enter_context(tc.tile_pool(name="expp", bufs=2))
        stkp = ctx.enter_context(tc.tile_pool(name="stkp", bufs=1))
        stage = ctx.enter_context(tc.tile_pool(name="stage", bufs=2))
        arp = ctx.enter_context(tc.tile_pool(name="arp", bufs=2))
        actp = ctx.enter_context(tc.tile_pool(name="actp", bufs=1))

        dram = ctx.enter_context(tc.tile_pool(name="dram", bufs=2, space="DRAM"))
        # persistent psum: norm sumsq / odd softmax sums (1 bank) + row
        # broadcasts (1 bank)
        pnorm = ctx.enter_context(tc.tile_pool(name="pnorm", bufs=1, space="PSUM"))

        # ---- persistent constants ----
        onesb = const.tile([128, 1], F16)
        nc.vector.memset(onesb[:], 1.0)
        onesr = const.tile([1, 128], F16)
        nc.vector.memset(onesr[:], 1.0)
        ones_lo = const.tile([1, 128], F16)
        nc.vector.memset(ones_lo[0:1, 0:64], 1.0)
        nc.vector.memset(ones_lo[0:1, 64:128], 0.0)
        ones_hi = const.tile([1, 128], F16)
        nc.vector.memset(ones_hi[0:1, 0:64], 0.0)
        nc.vector.memset(ones_hi[0:1, 64:128], 1.0)
        # small consts in ONE packed DMA, first on sync (scav's mask
        # matmuls can get scheduled early and head-block the PE queue on
        # these); big cos/sin tables on the gpsimd DGE queue
        cpk = const.tile([128, 704], F16)
        nc.sync.dma_start(cpk[:], cpack_in[:])
        step0 = cpk[:, 0:256]
        step1 = cpk[:, 256:512]
        nbi = cpk[:, 512:640]
        ident = cpk[:, 640:704]
        cosr = const.tile([128, S], F16)
        nc.gpsimd.dma_start(cosr[:], cos_in[:])
        sinr = const.tile([128, S], F16)
        nc.gpsimd.dma_start(sinr[:], sin_in[:])
        epsb = const.tile([1, 1], F32)
        nc.vector.memset(epsb[:], EPS)

        # residual stream + normalized copy, fp16, resident (big tiles)
        xtb = xtp.tile([128, KT * S], F16, tag="xtb", name="xtb")
        xbnb = xtp.tile([128, KT * S], F16, tag="xbnb", name="xbnb")

        def XT(k, st=None):
            if st is None:
                return xtb[:, k * S:(k + 1) * S]
            return xtb[:, k * S + st * HS:k * S + (st + 1) * HS]

        def XBN(k, st=None):
            if st is None:
                return xbnb[:, k * S:(k + 1) * S]
            return xbnb[:, k * S + st * HS:k * S + (st + 1) * HS]

        xtb3 = xtb.rearrange("p (k s) -> p k s", k=KT)
        xbnb3 = xbnb.rearrange("p (k s) -> p k s", k=KT)

        def load_x():
            # xbn0 in 4 contiguous 4-k-tile chunks so qkv(T0) starts after
            # the first chunk (subtile deps). xt goes on the gpsimd queue.
            for c in range(4):
                nc.sync.dma_start(
                    xbnb[:, 4 * c * S:(4 * c + 4) * S],
                    bass.AP(tensor=xbn0_in.tensor,
                            offset=xbn0_in.offset + 4 * c * 128 * S,
                            ap=[[S, 128], [128 * S, 4], [1, S]]))
            for st in range(2):
                nc.gpsimd.dma_start(
                    xtb3[:, :, st * HS:(st + 1) * HS],
                    bass.AP(tensor=xt_in.tensor, offset=xt_in.offset + st * HS,
                            ap=[[S, 128], [128 * S, KT], [1, HS]]))

        # ---------- per layer-token-half pieces ----------

        def residual_and_norm_half(pa, cc_out, st):
            """xt(st) += AR result; sumsq stats; r; xbn(st)."""
            stsl = slice(st * HS, (st + 1) * HS)
            ssum = pnorm.tile([1, HS], F32, tag="ssum", name="ssum")
            for q in range(4):
                ar_w = arp.tile([128, 4 * HS], F16, tag="ar")
                nc.sync.dma_start(
                    ar_w[:],
                    bass.AP(tensor=cc_out.tensor, offset=cc_out.offset + q * 4 * 128 * HS,
                            ap=[[HS, 128], [128 * HS, 4], [1, HS]]))
                for i in range(4):
                    k = q * 4 + i
                    nc.vector.tensor_add(XT(k, st), XT(k, st), ar_w[:, i * HS:(i + 1) * HS])
                    xsq = sq.tile([128, HS], F16, tag="xsq")
                    nc.vector.tensor_tensor(xsq[:], XT(k, st), XT(k, st), AluOpType.mult)
                    nc.tensor.matmul(ssum[:], onesb[:], xsq[:],
                                     start=(k == 0), stop=(k == KT - 1))
            rr16 = rp.tile([1, HS], F16, tag="rr16")
            nc.scalar.activation(rr16[:], ssum[:], AF.Rsqrt, bias=epsb[:],
                                 scale=1.0 / DM)
            rb_ps = pnorm.tile([128, HS], F32, tag="rbp", name="rb_ps")
            nc.tensor.matmul(rb_ps[:], onesr[:], rr16[:], start=True, stop=True)
            rb = rp.tile([128, HS], F16, tag="rb")
            nc.vector.tensor_copy(rb[:], rb_ps[:])
            for k in range(KT):
                nc.vector.tensor_tensor(XBN(k, st), XT(k, st), rb[:], AluOpType.mult)

        def rope_one(t, nrows, out, outrows, stsl):
            rot = ropet.tile([128, HS], F16, tag="rot")
            for h0 in range(0, nrows, 64):
                nc.sync.dma_start(rot[h0:h0 + 32, :], t[h0 + 32:h0 + 64, stsl])
                nc.sync.dma_start(rot[h0 + 32:h0 + 64, :], t[h0:h0 + 32, stsl])
            t1 = ropet.tile([128, HS], F16, tag="t1")
            nc.vector.tensor_tensor(t1[0:nrows, :], t[0:nrows, stsl],
                                    cosr[0:nrows, stsl], AluOpType.mult)
            t2 = ropet.tile([128, HS], F16, tag="t2")
            nc.vector.tensor_tensor(t2[0:nrows, :], rot[0:nrows, :],
                                    sinr[0:nrows, stsl], AluOpType.mult)
            nc.vector.tensor_add(out[outrows, stsl], t1[0:nrows, :], t2[0:nrows, :])

        def qkv_rope_half(pa, wqa, q01, q23, kv2, kt2, st):
            """q-pass, rope(q) emitted before kv-pass so it overlaps on DVE."""
            stsl = slice(st * HS, (st + 1) * HS)
            pq0 = pa.tile([128, HS], F32, tag="pq0", name="pq0")
            pq1 = pa.tile([128, HS], F32, tag="pq1", name="pq1")
            for k in range(KT):
                w0 = k * 384
                st_, sp_ = (k == 0), (k == KT - 1)
                nc.tensor.matmul(pq0[:], wqa[:, w0:w0 + 128], XBN(k, st),
                                 start=st_, stop=sp_)
                nc.tensor.matmul(pq1[:], wqa[:, w0 + 128:w0 + 256], XBN(k, st),
                                 start=st_, stop=sp_)
            nc.vector.tensor_copy(q01[:, stsl], pq0[:])
            nc.vector.tensor_copy(q23[:, stsl], pq1[:])
            rope_one(q01, 128, q01, slice(0, 128), stsl)
            rope_one(q23, 128, q23, slice(0, 128), stsl)
            # kv-pass: one psum; its matmuls overlap the q-rope DVE work
            pkv = pa.tile([128, HS], F32, tag="sps0", name="pkv")
            for k in range(KT):
                w0 = k * 384
                nc.tensor.matmul(pkv[:], wqa[:, w0 + 256:w0 + 384], XBN(k, st),
                                 start=(k == 0), stop=(k == KT - 1))
            # k rows first so rope-k starts while v evacuates
            nc.scalar.copy(kv2[0:64, stsl], pkv[0:64, :])
            nc.scalar.copy(kv2[64:128, stsl], pkv[64:128, :])
            rope_one(kv2, 64, kt2, slice(0, 64), stsl)
            nc.sync.dma_start(kt2[64:128, stsl], kt2[0:64, stsl])

        def vtp(pa, kv2, va, sj):
            tp = pa.tile([128, 64], F16, tag=f"aps{sj % 2}", name="tp")
            nc.tensor.transpose(tp[:], kv2[64:128, sj * 128:(sj + 1) * 128],
                                ident[64:128, :])
            v = vap.tile([128, 65], F16, tag=f"va{sj}", name=f"va{sj}")
            nc.vector.tensor_copy(v[:, 0:64], tp[:])
            nc.vector.memset(v[:, 64:65], 1.0)
            va.append(v)

        def attn_half(pa, q01, q23, kt2, kv2, va, stk0, stk1, sinv_sb,
                      wo0, wo1, cc_in, st):
            """scores+exp+av, per-block normalize, and per-block wo for the
            query blocks of half st (it in 2st..2st+1).

            it-outer so both parities of a query block finish together; the
            block is then normalized and its wo contribution computed while
            the next block's scores run. v-transposes for the later key
            blocks are emitted mid-stream to fill the softmax warmup.
            """
            wo_tags = ("pq0", "pq1", "sps0", "sps1")
            stgs = [stage.tile([128, 2048], F16, tag=f"wostg{j % 2}",
                               name=f"stg{j}") for j in range(4)]
            jp_ctr = 0
            for it in range(2 * st, 2 * st + 2):
                for parity in range(2):
                    rows = slice(64 * parity, 64 * parity + 64)
                    odd = parity == 1
                    isl = slice(it * 256, (it + 1) * 256)
                    aps = [pa.tile([128, 512], F32, tag=f"aps{m}", name=f"aps{m}")
                           for m in range(2)]
                    for jp in range(it + 1):
                        diag = jp == it
                        tg = ("pq0", "pq1") if jp_ctr % 2 else ("sps0", "sps1")
                        jp_ctr += 1
                        sps = [pa.tile([128, 512], F32, tag=tg[m], name=f"sps{m}")
                               for m in range(2)]
                        for half in range(2):
                            j = 2 * jp + half
                            ssl = slice(half * 256, half * 256 + 256)
                            if diag:
                                for m in range(2):
                                    nc.tensor.matmul(sps[m][:, ssl], nbi[:],
                                                     (step0, step1)[half][:],
                                                     start=True, stop=False)
                            for m, qt in enumerate((q01, q23)):
                                nc.tensor.matmul(sps[m][:, ssl],
                                                 kt2[rows, j * 128:(j + 1) * 128],
                                                 qt[rows, isl],
                                                 start=not diag, stop=True)
                        es = []
                        for m in range(2):
                            e = expp.tile([128, 512], F16, tag=f"e{m}", name=f"e{m}")
                            nc.scalar.activation(e[:], sps[m][:], AF.Exp)
                            es.append(e)
                        if it == 2 * st and parity == 0 and jp == 0:
                            # transposes for the later key blocks fill the
                            # scores->exp->av pipeline warmup with PE work
                            vtp(pa, kv2, va, 4 * st + 2)
                            vtp(pa, kv2, va, 4 * st + 3)
                        first, last = (jp == 0), (jp == it)
                        for half in range(2):
                            esl = slice(half * 256, half * 256 + 256)
                            vsl = slice(0, 64) if odd else slice(0, 65)
                            orows = slice(64, 128) if odd else slice(0, 65)
                            for m in range(2):
                                # odd parity: av (rows 64:128) and key-sums
                                # (row 0) share a bank as two accumulation
                                # groups on disjoint partitions — PSUM
                                # pending-zero tracking is per-partition.
                                nc.tensor.matmul(aps[m][orows, 0:256],
                                                 va[2 * jp + half][:, vsl], es[m][:, esl],
                                                 start=(first and half == 0),
                                                 stop=(last and half == 1))
                            if odd:
                                for m in range(2):
                                    nc.tensor.matmul(
                                        aps[m][0:1, 0:256],
                                        onesb[:], es[m][:, esl],
                                        start=(first and half == 0),
                                        stop=(last and half == 1))
                    arows = slice(64, 128) if odd else slice(0, 64)
                    heads = (parity, parity + 2)
                    for m in range(2):
                        h = heads[m]
                        srow = (aps[m][0:1, 0:256] if odd
                                else aps[m][64:65, 0:256])
                        nc.vector.reciprocal(
                            sinv_sb[0:1, h * S + it * 256:h * S + it * 256 + 256], srow)
                        nc.vector.tensor_copy((stk0, stk1)[m][arows, isl],
                                              aps[m][arows, 0:256])
                # normalize this 256-col query block: 1/sums broadcast to the
                # two 64-row head blocks via K=1 outer products, multiplied
                # straight out of PSUM.
                for t, h0, h1 in ((stk0, 0, 1), (stk1, 2, 3)):
                    rb_ps = pnorm.tile([128, 256], F32, tag="rbp", name="sinv_ps")
                    nc.tensor.matmul(rb_ps[:], ones_lo[:],
                                     sinv_sb[0:1, h0 * S + it * 256:h0 * S + it * 256 + 256],
                                     start=True, stop=False)
                    nc.tensor.matmul(rb_ps[:], ones_hi[:],
                                     sinv_sb[0:1, h1 * S + it * 256:h1 * S + it * 256 + 256],
                                     start=False, stop=True)
                    nc.vector.tensor_tensor(t[:, isl], t[:, isl], rb_ps[:],
                                            AluOpType.mult)
                # wo contribution of this query block
                ito = it - 2 * st
                for dmm in range(16):
                    dsl = slice(dmm * 128, (dmm + 1) * 128)
                    wops = pa.tile([128, 256], F32, tag=wo_tags[dmm % 4], name="wops")
                    nc.tensor.matmul(wops[:], wo0[:, dsl], stk0[:, isl],
                                     start=True, stop=False)
                    nc.tensor.matmul(wops[:], wo1[:, dsl], stk1[:, isl],
                                     start=False, stop=True)
                    osl = slice((dmm % 4) * 512 + ito * 256,
                                (dmm % 4) * 512 + ito * 256 + 256)
                    if dmm % 2 == 0:
                        nc.vector.tensor_copy(stgs[dmm // 4][:, osl], wops[:])
                    else:
                        nc.scalar.copy(stgs[dmm // 4][:, osl], wops[:])
            for j in range(4):
                nc.scalar.dma_start(
                    bass.AP(tensor=cc_in.tensor,
                            offset=cc_in.offset + j * 4 * 128 * HS,
                            ap=[[HS, 128], [128 * HS, 4], [1, HS]]),
                    stgs[j][:])

        def gu_half(pm, l, prod, st):
            for fmh in range(2):
                fms = [4 * fmh + j for j in range(4)]
                for phase, w_in in (("g", wg_in), ("u", wu_in)):
                    ps = {}
                    for fm in fms:
                        ps[fm] = pm.tile([128, HS], F32, tag=f"m{fm % 4}",
                                         name=f"m{phase}{fm}")
                    for g in range(8):
                        wt = wbig.tile([128, 1024], F16, tag="wgu", name="wgu", bufs=4)
                        nc.scalar.dma_start(
                            wt[:],
                            bass.AP(tensor=w_in.tensor,
                                    offset=(w_in.offset + l * DM * FFS + g * 2 * 128 * FFS
                                            + fmh * 512),
                                    ap=[[FFS, 128], [128 * FFS, 2], [1, 512]]))
                        for i in range(2):
                            k = g * 2 + i
                            for fm in fms:
                                wsl = wt[:, i * 512 + (fm % 4) * 128:
                                         i * 512 + (fm % 4 + 1) * 128]
                                nc.tensor.matmul(ps[fm][:], wsl, XBN(k, st),
                                                 start=(k == 0), stop=(k == KT - 1))
                    if phase == "g":
                        sil = {}
                        for fm in fms:
                            t = actp.tile([128, HS], F16, tag=f"sil{fm % 4}",
                                          name=f"sil{fm}")
                            nc.scalar.activation(t[:], ps[fm][:], AF.Silu)
                            sil[fm] = t
                    else:
                        for fm in fms:
                            t = actp.tile([128, HS], F16, tag=f"prod{fm}",
                                          name=f"prod{fm}")
                            nc.vector.tensor_tensor(t[:], sil[fm][:], ps[fm][:],
                                                    AluOpType.mult)
                            prod[fm] = t

        def down_half(pm, l, prod, cc_in, st, last):
            stsl = slice(st * HS, (st + 1) * HS)
            for dmg in range(4):
                dps = {}
                for d in range(4):
                    dps[d] = pm.tile([128, HS], F32, tag=f"m{d}", name=f"md{d}")
                for gg in range(2):
                    wdt = wbig.tile([128, 2048], F16, tag="wdt", name="wdt", bufs=3)
                    nc.scalar.dma_start(
                        wdt[:],
                        bass.AP(tensor=wd_in.tensor,
                                offset=(wd_in.offset + l * FFS * DM + gg * 4 * 128 * DM
                                        + dmg * 512),
                                ap=[[DM, 128], [128 * DM, 4], [1, 512]]))
                    for i2 in range(4):
                        fk = gg * 4 + i2
                        for d in range(4):
                            wsl = wdt[:, i2 * 512 + d * 128:i2 * 512 + (d + 1) * 128]
                            nc.tensor.matmul(dps[d][:], wsl, prod[fk][:],
                                             start=(fk == 0), stop=(fk == FMT - 1))
                for dp in range(2):
                    stg = stage.tile([128, 1024], F16, tag="dstg")
                    for i in range(2):
                        d = dp * 2 + i
                        kk = dmg * 4 + d
                        osl = slice(i * HS, (i + 1) * HS)
                        if last:
                            nc.vector.scalar_tensor_tensor(
                                stg[:, osl], XT(kk, st), 1.0 / NCORES, dps[d][:],
                                AluOpType.mult, AluOpType.add)
                        elif i == 0:
                            nc.vector.tensor_copy(stg[:, osl], dps[d][:])
                        else:
                            nc.scalar.copy(stg[:, osl], dps[d][:])
                    dst = y_out if last else cc_in
                    if last:
                        dstap = bass.AP(
                            tensor=dst.tensor,
                            offset=dst.offset + (dmg * 4 + dp * 2) * 128 * S + st * HS,
                            ap=[[S, 128], [128 * S, 2], [1, HS]])
                    else:
                        dstap = bass.AP(
                            tensor=dst.tensor,
                            offset=dst.offset + (dmg * 4 + dp * 2) * 128 * HS,
                            ap=[[HS, 128], [128 * HS, 2], [1, HS]])
                    nc.scalar.dma_start(dstap, stg[:])

        def allreduce(cc_in, cc_out):
            nc.gpsimd.collective_compute(
                "AllReduce", AluOpType.add,
                replica_groups=[list(range(NCORES))],
                ins=[cc_in[:].opt()], outs=[cc_out[:].opt()])

        def cc_pair(tagbase, st):
            cc_in = dram.tile([DM, HS], F16, tag=f"{tagbase}i{st}", name="cc_in")
            if skip_cc:
                return cc_in, cc_in
            cc_out = dram.tile([DM, HS], F16, tag=f"{tagbase}o{st}", name="cc_out",
                               addr_space="Shared")
            return cc_in, cc_out

        # ---------- main program ----------
        for _ in range(reps):
            load_x()
            # mlp-norm carried across the layer boundary
            pend_mlp_cc = [None, None]
            for l in range(L):
                ctx_a = ExitStack()
                pa = ctx_a.enter_context(tc.tile_pool(name="pa", bufs=1, space="PSUM"))
                wqa = wqp.tile([128, KT * 384], F16, tag="wqa", name="wqa")
                for g in range(4):
                    nc.scalar.dma_start(
                        wqa[:, g * 4 * 384:(g + 1) * 4 * 384],
                        bass.AP(tensor=wqkv_in.tensor,
                                offset=wqkv_in.offset + l * DM * 384 + g * 4 * 128 * 384,
                                ap=[[384, 128], [128 * 384, 4], [1, 384]]))
                wo0 = wob.tile([128, DM], F16, tag="wo0")
                nc.sync.dma_start(wo0[:], wo_in[l, 0:128, :])
                wo1 = wob.tile([128, DM], F16, tag="wo1")
                nc.sync.dma_start(wo1[:], wo_in[l, 128:256, :])

                q01 = attn_sb.tile([128, S], F16, tag="q01")
                q23 = attn_sb.tile([128, S], F16, tag="q23")
                kv2 = attn_sb.tile([128, S], F16, tag="kv2")
                kt2 = attn_sb.tile([128, S], F16, tag="kt2")
                stk0 = stkp.tile([128, S], F16, tag="stk0")
                stk1 = stkp.tile([128, S], F16, tag="stk1")
                sinv_sb = stkp.tile([1, 4 * S], F16, tag="sinv_sb")
                va = []
                cc_a = [cc_pair("a", st) for st in range(2)]

                for st in range(2):
                    if pend_mlp_cc[st] is not None:
                        residual_and_norm_half(pa, pend_mlp_cc[st], st)
                        pend_mlp_cc[st] = None
                    qkv_rope_half(pa, wqa, q01, q23, kv2, kt2, st)
                    vtp(pa, kv2, va, 4 * st)
                    vtp(pa, kv2, va, 4 * st + 1)
                    attn_half(pa, q01, q23, kt2, kv2, va, stk0, stk1, sinv_sb,
                              wo0, wo1, cc_a[st][0], st)
                    if not skip_cc:
                        allreduce(cc_a[st][0], cc_a[st][1])

                if debug_stage == f"stk{l}":
                    for ti, t in enumerate((stk0, stk1)):
                        nc.sync.dma_start(y_out[ti * 128:(ti + 1) * 128, :], t[:])
                    ctx_a.close()
                    break
                ctx_a.close()

                last = (l == L - 1 and debug_stage is None)
                ctx_m = ExitStack()
                pm = ctx_m.enter_context(tc.tile_pool(name="pm", bufs=1, space="PSUM"))
                cc_m = [None if last else cc_pair("m", st) for st in range(2)]
                for st in range(2):
                    residual_and_norm_half(pm, cc_a[st][1], st)
                    prod = {}
                    gu_half(pm, l, prod, st)
                    down_half(pm, l, prod, None if last else cc_m[st][0], st, last)
                    if not last and not skip_cc:
                        allreduce(cc_m[st][0], cc_m[st][1])
                    if not last:
                        pend_mlp_cc[st] = cc_m[st][1]
                ctx_m.close()
                if debug_stage == f"attn{l}" or debug_stage == f"mlp{l}":
                    for k in range(KT):
                        nc.sync.dma_start(y_out[k * 128:(k + 1) * 128, :], XT(k))
                    break

    nc.compile()
    return nc


def make_inputs(input_ids, embed, wq, wk, wv, wo, wgate, wup, wdown, ln1, ln2):
    """host-side prep: embedding gather, layer0-norm, shard + fold gains."""
    f32 = np.float32
    f16 = np.float16
    x = np.asarray(embed, f32)[np.asarray(input_ids)[0]]      # (S, DM)
    xt = np.ascontiguousarray(x.T)                            # (DM, S)
    r0 = 1.0 / np.sqrt(np.mean(xt * xt, axis=0) + EPS)        # (S,)
    xbn0 = (xt * r0[None, :]).astype(f16)

    inv_freq = 1.0 / (THETA ** (np.arange(0, D, 2, dtype=f32) / D))
    freqs = np.arange(S, dtype=f32)[:, None] * inv_freq[None, :]    # (S, 32)
    emb = np.concatenate([freqs, freqs], axis=1)                    # (S, D)
    cosT = np.cos(emb).T.astype(f32)                                # (D, S)
    sinT = np.sin(emb).T.astype(f32)
    sinT_signed = sinT.copy()
    sinT_signed[: D // 2] *= -1.0
    cos_rep = np.concatenate([cosT, cosT], axis=0).astype(f16)      # (128, S)
    sin_rep = np.concatenate([sinT_signed, sinT_signed], axis=0).astype(f16)

    step = np.zeros((2, 128, 256), f16)
    jj = np.arange(128)[:, None]
    ii = np.arange(256)[None, :]
    for o in range(2):
        step[o] = (128 * o + jj > ii).astype(f16)
    negbigI = (NEGBIG * np.eye(128, dtype=f32)).astype(f16)
    ident = np.concatenate([np.eye(64), np.eye(64)], axis=0).astype(f16)
    cpack = np.concatenate([step[0], step[1], negbigI, ident], axis=1).astype(f16)

    scale_q = 1.0 / math.sqrt(D)
    in_maps = []
    for c in range(NCORES):
        wqkv = np.empty((L, DM, 384), f32)
        wo_c = np.empty((L, QC, DM), f32)
        wg_c = np.empty((L, DM, FFS), f32)
        wu_c = np.empty((L, DM, FFS), f32)
        wd_c = np.empty((L, FFS, DM), f32)
        for l in range(L):
            g1 = np.asarray(ln1[l], f32)[:, None]
            g2 = np.asarray(ln2[l], f32)[:, None]
            wqkv[l, :, :QC] = np.asarray(wq[l], f32)[:, c * QC:(c + 1) * QC] * g1 * scale_q
            wqkv[l, :, QC:QC + D] = np.asarray(wk[l], f32)[:, c * D:(c + 1) * D] * g1
            wqkv[l, :, QC + D:] = np.asarray(wv[l], f32)[:, c * D:(c + 1) * D] * g1
            wo_c[l] = np.asarray(wo[l], f32)[c * QC:(c + 1) * QC, :]
            wg_c[l] = np.asarray(wgate[l], f32)[:, c * FFS:(c + 1) * FFS] * g2
            wu_c[l] = np.asarray(wup[l], f32)[:, c * FFS:(c + 1) * FFS] * g2
            wd_c[l] = np.asarray(wdown[l], f32)[c * FFS:(c + 1) * FFS, :]
        in_maps.append({
            "xt16": xt.astype(f16), "xbn0": xbn0, "wqkv": wqkv.astype(f16),
            "wo": wo_c.astype(f16), "wg": wg_c.astype(f16), "wu": wu_c.astype(f16),
            "wd": wd_c.astype(f16), "cosr": cos_rep, "sinr": sin_rep,
            "cpack": cpack,
        })
    return in_maps


_NC_CACHE = {}


def kernel(**inputs) -> np.ndarray:
    if 1 not in _NC_CACHE:
        _NC_CACHE[1] = build(reps=1)
    nc = _NC_CACHE[1]
    in_maps = make_inputs(**inputs)
    res = run_bass_kernel_spmd(nc, in_maps, list(range(NCORES)))
    y = np.zeros((DM, S), np.float64)
    for c in range(NCORES):
        y += res.results[c]["y"].astype(np.float64)
    return np.ascontiguousarray(y.T.astype(np.float32)).reshape(B, S, DM)


# revision 36
# speedup vs baseline: 1.2757x; 1.1426x over previous
"""Llama trunk (2 layers, before final norm) on 8 trn2 cores.

Sharding: Megatron tensor-parallel over 8 cores.
  - attention: 4 q-heads + 1 kv-head per core (GQA group stays local)
  - MLP: 1024 of 8192 intermediate dims per core
  - residual stream xt kept in fp16, transposed [DM(part), S(free)]

v2: token-half software pipeline. All per-token phases (qkv, rope, wo,
norm, MLP) are processed in two 512-column halves T0/T1, and the three
on-device AllReduces are split into six half-payload AllReduces, each
overlapped with the other half's compute (collectives run on dedicated
SDMA/CCE silicon, so the PE keeps streaming through them). Attention is
causal, so T0 queries only need T0 keys and the attention block also
pipelines by half. Other changes vs v1:
  - softmax 1/sum is broadcast to the 64-row head blocks with two K=1
    outer-product matmuls into PSUM (ones_lo/ones_hi stationaries)
    instead of a DRAM round-trip.
  - qkv is two passes (q-pass: 2 psums, kv-pass: 1) so attention fits
    the 8 PSUM banks together with wo / norm psums; weights stay in
    SBUF across both passes and both halves.
  - odd-parity softmax key-sums go to one packed PSUM bank (partition
    0 / 32 via tile_position) instead of two.
"""
import math
from contextlib import ExitStack

import numpy as np

import concourse.bass as bass
import concourse.tile as tile
from concourse import bacc, mybir
from concourse.alu_op_type import AluOpType
from concourse.bass_utils import run_bass_kernel_spmd

L, H, KVH, D = 2, 32, 8, 64
DM, FF = 2048, 8192
B, S = 1, 1024
EPS, THETA = 1e-5, 10000.0
NCORES = 8
QH = H // NCORES            # 4 q heads per core
QC = QH * D                 # 256 q cols per core
FFS = FF // NCORES          # 1024 ff dims per core
KT = DM // 128              # 16 contraction tiles over DM
FMT = FFS // 128            # 8 f tiles
HS = S // 2                 # 512 token half
NEGBIG = -30000.0

F32 = mybir.dt.float32
F16 = mybir.dt.float16
AF = mybir.ActivationFunctionType


def build(reps=1, debug_stage=None, skip_cc=False, cc_nowait=False):
    nc = bacc.Bacc(None, target_bir_lowering=False, debug=False, num_devices=NCORES)
    xt_in = nc.dram_tensor("xt16", [DM, S], F16, kind="ExternalInput").ap()
    xbn0_in = nc.dram_tensor("xbn0", [DM, S], F16, kind="ExternalInput").ap()
    wqkv_in = nc.dram_tensor("wqkv", [L, DM, 384], F16, kind="ExternalInput").ap()
    wo_in = nc.dram_tensor("wo", [L, QC, DM], F16, kind="ExternalInput").ap()
    wg_in = nc.dram_tensor("wg", [L, DM, FFS], F16, kind="ExternalInput").ap()
    wu_in = nc.dram_tensor("wu", [L, DM, FFS], F16, kind="ExternalInput").ap()
    wd_in = nc.dram_tensor("wd", [L, FFS, DM], F16, kind="ExternalInput").ap()
    cos_in = nc.dram_tensor("cosr", [128, S], F16, kind="ExternalInput").ap()
    sin_in = nc.dram_tensor("sinr", [128, S], F16, kind="ExternalInput").ap()
    cpack_in = nc.dram_tensor("cpack", [128, 704], F16, kind="ExternalInput").ap()
    y_out = nc.dram_tensor("y", [DM, S], F16, kind="ExternalOutput").ap()

    with tile.TileContext(nc) as tc, ExitStack() as ctx, \
            nc.allow_low_precision(reason="deliberate fp16 pipeline, tol 2e-2"):
        const = ctx.enter_context(tc.tile_pool(name="const", bufs=1))
        xtp = ctx.enter_context(tc.tile_pool(name="xtp", bufs=1))
        wqp = ctx.enter_context(tc.tile_pool(name="wqp", bufs=1))
        wob = ctx.enter_context(tc.tile_pool(name="wob", bufs=1))
        wbig = ctx.enter_context(tc.tile_pool(name="wbig", bufs=2))
        sq = ctx.enter_context(tc.tile_pool(name="sq", bufs=2))
        rp = ctx.enter_context(tc.tile_pool(name="rp", bufs=2))
        attn_sb = ctx.enter_context(tc.tile_pool(name="attn_sb", bufs=1))
        ropet = ctx.enter_context(tc.tile_pool(name="ropet", bufs=2))
        vap = ctx.enter_context(tc.tile_pool(name="vap", bufs=1))
        expp = ctx.enter_context(tc.tile_pool(name="expp", bufs=2))
        stkp = ctx.enter_context(tc.tile_pool(name="stkp", bufs=1))
        stage = ctx.enter_context(tc.tile_pool(name="stage", bufs=2))
        arp = ctx.enter_context(tc.tile_pool(name="arp", bufs=2))
        actp = ctx.enter_context(tc.tile_pool(name="actp", bufs=1))

        dram = ctx.enter_context(tc.tile_pool(name="dram", bufs=2, space="DRAM"))
        # persistent psum: norm sumsq / odd softmax sums (1 bank) + row
        # broadcasts (1 bank)
        pnorm = ctx.enter_context(tc.tile_pool(name="pnorm", bufs=1, space="PSUM"))

        # ---- persistent constants ----
        onesb = const.tile([128, 1], F16)
        nc.vector.memset(onesb[:], 1.0)
        onesr = const.tile([1, 128], F16)
        nc.vector.memset(onesr[:], 1.0)
        ones_lo = const.tile([1, 128], F16)
        nc.vector.memset(ones_lo[0:1, 0:64], 1.0)
        nc.vector.memset(ones_lo[0:1, 64:128], 0.0)
        ones_hi = const.tile([1, 128], F16)
        nc.vector.memset(ones_hi[0:1, 0:64], 0.0)
        nc.vector.memset(ones_hi[0:1, 64:128], 1.0)
        # small consts in ONE packed DMA, first on sync (scav's mask
        # matmuls can get scheduled early and head-block the PE queue on
        # these); big cos/sin tables on the gpsimd DGE queue
        cpk = const.tile([128, 704], F16)
        nc.sync.dma_start(cpk[:], cpack_in[:])
        step0 = cpk[:, 0:256]
        step1 = cpk[:, 256:512]
        nbi = cpk[:, 512:640]
        ident = cpk[:, 640:704]
        cosr = const.tile([128, S], F16)
        nc.gpsimd.dma_start(cosr[:], cos_in[:])
        sinr = const.tile([128, S], F16)
        nc.gpsimd.dma_start(sinr[:], sin_in[:])
        epsb = const.tile([1, 1], F32)
        nc.vector.memset(epsb[:], EPS)

        # residual stream + normalized copy, fp16, resident (big tiles)
        xtb = xtp.tile([128, KT * S], F16, tag="xtb", name="xtb")
        xbnb = xtp.tile([128, KT * S], F16, tag="xbnb", name="xbnb")

        def XT(k, st=None):
            if st is None:
                return xtb[:, k * S:(k + 1) * S]
            return xtb[:, k * S + st * HS:k * S + (st + 1) * HS]

        def XBN(k, st=None):
            if st is None:
                return xbnb[:, k * S:(k + 1) * S]
            return xbnb[:, k * S + st * HS:k * S + (st + 1) * HS]

        xtb3 = xtb.rearrange("p (k s) -> p k s", k=KT)
        xbnb3 = xbnb.rearrange("p (k s) -> p k s", k=KT)

        def load_x():
            # xbn0 in 4 contiguous 4-k-tile chunks so qkv(T0) starts after
            # the first chunk (subtile deps). xt goes on the gpsimd queue.
            for c in range(4):
                nc.sync.dma_start(
                    xbnb[:, 4 * c * S:(4 * c + 4) * S],
                    bass.AP(tensor=xbn0_in.tensor,
                            offset=xbn0_in.offset + 4 * c * 128 * S,
                            ap=[[S, 128], [128 * S, 4], [1, S]]))
            for st in range(2):
                nc.gpsimd.dma_start(
                    xtb3[:, :, st * HS:(st + 1) * HS],
                    bass.AP(tensor=xt_in.tensor, offset=xt_in.offset + st * HS,
                            ap=[[S, 128], [128 * S, KT], [1, HS]]))

        # ---------- per layer-token-half pieces ----------

        def residual_and_norm_half(pa, cc_out, st):
            """cc_out already holds x_new = residual + block output (the x/8
            fold happens in the stage evacuations, so the AllReduce sums it).
            Here: sumsq stats; r; xbn(st); lazy copy of x_new into xtb."""
            ssum = pnorm.tile([1, HS], F32, tag="ssum", name="ssum")
            queues = (nc.sync, nc.scalar, nc.gpsimd)
            ar_ws = []
            for q in range(8):
                ar_w = arp.tile([128, 2 * HS], F16, tag=f"ar{q}", bufs=1, name="ar")
                queues[q % 3].dma_start(
                    ar_w[:],
                    bass.AP(tensor=cc_out.tensor, offset=cc_out.offset + q * 2 * 128 * HS,
                            ap=[[HS, 128], [128 * HS, 2], [1, HS]]))
                ar_ws.append(ar_w)
            def ARW(k):
                return ar_ws[k // 2][:, (k % 2) * HS:(k % 2 + 1) * HS]
            for k in range(KT):
                xsq = sq.tile([128, HS], F16, tag="xsq")
                nc.vector.tensor_tensor(xsq[:], ARW(k), ARW(k), AluOpType.mult)
                nc.tensor.matmul(ssum[:], onesb[:], xsq[:],
                                 start=(k == 0), stop=(k == KT - 1))
            rs = rp.tile([1, HS], F32, tag="rs")
            nc.scalar.activation(rs[:], ssum[:], AF.Sqrt, bias=epsb[:], scale=1.0 / DM)
            rrf = rp.tile([1, HS], F32, tag="rrf")
            nc.vector.reciprocal(rrf[:], rs[:])
            rr16 = rp.tile([1, HS], F16, tag="rr16")
            nc.vector.tensor_copy(rr16[:], rrf[:])
            rb_ps = pnorm.tile([128, HS], F32, tag="rbp", name="rb_ps")
            nc.tensor.matmul(rb_ps[:], onesr[:], rr16[:], start=True, stop=True)
            rb = rp.tile([128, HS], F16, tag="rb")
            nc.vector.tensor_copy(rb[:], rb_ps[:])
            for k in range(KT):
                nc.vector.tensor_tensor(XBN(k, st), ARW(k), rb[:], AluOpType.mult)
            # x_new -> xtb off the critical path (needed by the next block's
            # stage fold, ~40us later)
            for k in range(KT):
                nc.vector.tensor_copy(XT(k, st), ARW(k))

        def rope_one(t, nrows, out, outrows, stsl):
            rot = ropet.tile([128, HS], F16, tag="rot")
            for h0 in range(0, nrows, 64):
                nc.sync.dma_start(rot[h0:h0 + 32, :], t[h0 + 32:h0 + 64, stsl])
                nc.sync.dma_start(rot[h0 + 32:h0 + 64, :], t[h0:h0 + 32, stsl])
            t1 = ropet.tile([128, HS], F16, tag="t1")
            nc.vector.tensor_tensor(t1[0:nrows, :], t[0:nrows, stsl],
                                    cosr[0:nrows, stsl], AluOpType.mult)
            t2 = ropet.tile([128, HS], F16, tag="t2")
            nc.vector.tensor_tensor(t2[0:nrows, :], rot[0:nrows, :],
                                    sinr[0:nrows, stsl], AluOpType.mult)
            nc.vector.tensor_add(out[outrows, stsl], t1[0:nrows, :], t2[0:nrows, :])

        def qkv_rope_half(pa, wqa, q01, q23, kv2, kt2, st):
            """q-pass, rope(q) emitted before kv-pass so it overlaps on DVE."""
            stsl = slice(st * HS, (st + 1) * HS)
            pq0 = pa.tile([128, HS], F32, tag="pq0", name="pq0")
            pq1 = pa.tile([128, HS], F32, tag="pq1", name="pq1")
            for k in range(KT):
                w0 = k * 384
                st_, sp_ = (k == 0), (k == KT - 1)
                nc.tensor.matmul(pq0[:], wqa[:, w0:w0 + 128], XBN(k, st),
                                 start=st_, stop=sp_)
                nc.tensor.matmul(pq1[:], wqa[:, w0 + 128:w0 + 256], XBN(k, st),
                                 start=st_, stop=sp_)
            nc.vector.tensor_copy(q01[:, stsl], pq0[:])
            nc.vector.tensor_copy(q23[:, stsl], pq1[:])
            rope_one(q01, 128, q01, slice(0, 128), stsl)
            rope_one(q23, 128, q23, slice(0, 128), stsl)
            # kv-pass: one psum; its matmuls overlap the q-rope DVE work
            pkv = pa.tile([128, HS], F32, tag="sps0", name="pkv")
            for k in range(KT):
                w0 = k * 384
                nc.tensor.matmul(pkv[:], wqa[:, w0 + 256:w0 + 384], XBN(k, st),
                                 start=(k == 0), stop=(k == KT - 1))
            # k rows first so rope-k starts while v evacuates
            nc.scalar.copy(kv2[0:64, stsl], pkv[0:64, :])
            nc.scalar.copy(kv2[64:128, stsl], pkv[64:128, :])
            rope_one(kv2, 64, kt2, slice(0, 64), stsl)
            nc.sync.dma_start(kt2[64:128, stsl], kt2[0:64, stsl])

        def vtp(pa, kv2, va, sj):
            tp = pa.tile([128, 64], F16, tag=f"aps{sj % 2}", name="tp")
            nc.tensor.transpose(tp[:], kv2[64:128, sj * 128:(sj + 1) * 128],
                                ident[64:128, :])
            v = vap.tile([128, 65], F16, tag=f"va{sj}", name=f"va{sj}")
            nc.vector.tensor_copy(v[:, 0:64], tp[:])
            nc.vector.memset(v[:, 64:65], 1.0)
            va.append(v)

        def attn_half(pa, q01, q23, kt2, kv2, va, stk0, stk1, sinv_sb,
                      wo0, wo1, cc_in, st):
            """scores+exp+av, per-block normalize, and per-block wo for the
            query blocks of half st (it in 2st..2st+1).

            it-outer so both parities of a query block finish together; the
            block is then normalized and its wo contribution computed while
            the next block's scores run. v-transposes for the later key
            blocks are emitted mid-stream to fill the softmax warmup.
            """
            wo_tags = ("pq0", "pq1", "sps0", "sps1")
            stgs = [stage.tile([128, 2048], F16, tag=f"wostg{j % 2}",
                               name=f"stg{j}") for j in range(4)]
            jp_ctr = 0
            for it in range(2 * st, 2 * st + 2):
                for parity in range(2):
                    rows = slice(64 * parity, 64 * parity + 64)
                    odd = parity == 1
                    isl = slice(it * 256, (it + 1) * 256)
                    aps = [pa.tile([128, 512], F32, tag=f"aps{m}", name=f"aps{m}")
                           for m in range(2)]
                    for jp in range(it + 1):
                        diag = jp == it
                        tg = ("pq0", "pq1") if jp_ctr % 2 else ("sps0", "sps1")
                        jp_ctr += 1
                        sps = [pa.tile([128, 512], F32, tag=tg[m], name=f"sps{m}")
                               for m in range(2)]
                        for half in range(2):
                            j = 2 * jp + half
                            ssl = slice(half * 256, half * 256 + 256)
                            if diag:
                                for m in range(2):
                                    nc.tensor.matmul(sps[m][:, ssl], nbi[:],
                                                     (step0, step1)[half][:],
                                                     start=True, stop=False)
                            for m, qt in enumerate((q01, q23)):
                                nc.tensor.matmul(sps[m][:, ssl],
                                                 kt2[rows, j * 128:(j + 1) * 128],
                                                 qt[rows, isl],
                                                 start=not diag, stop=True)
                        es = []
                        for m in range(2):
                            e = expp.tile([128, 512], F16, tag=f"e{m}", name=f"e{m}")
                            nc.scalar.activation(e[:], sps[m][:], AF.Exp)
                            es.append(e)
                        if it == 2 * st and parity == 0 and jp == 0:
                            # transposes for the later key blocks fill the
                            # scores->exp->av pipeline warmup with PE work
                            vtp(pa, kv2, va, 4 * st + 2)
                            vtp(pa, kv2, va, 4 * st + 3)
                        first, last = (jp == 0), (jp == it)
                        for half in range(2):
                            esl = slice(half * 256, half * 256 + 256)
                            vsl = slice(0, 64) if odd else slice(0, 65)
                            orows = slice(64, 128) if odd else slice(0, 65)
                            for m in range(2):
                                # odd parity: av (rows 64:128) and key-sums
                                # (row 0) share a bank as two accumulation
                                # groups on disjoint partitions — PSUM
                                # pending-zero tracking is per-partition.
                                nc.tensor.matmul(aps[m][orows, 0:256],
                                                 va[2 * jp + half][:, vsl], es[m][:, esl],
                                                 start=(first and half == 0),
                                                 stop=(last and half == 1))
                            if odd:
                                for m in range(2):
                                    nc.tensor.matmul(
                                        aps[m][0:1, 0:256],
                                        onesb[:], es[m][:, esl],
                                        start=(first and half == 0),
                                        stop=(last and half == 1))
                    arows = slice(64, 128) if odd else slice(0, 64)
                    heads = (parity, parity + 2)
                    for m in range(2):
                        h = heads[m]
                        srow = (aps[m][0:1, 0:256] if odd
                                else aps[m][64:65, 0:256])
                        nc.vector.reciprocal(
                            sinv_sb[0:1, h * S + it * 256:h * S + it * 256 + 256], srow)
                        nc.vector.tensor_copy((stk0, stk1)[m][arows, isl],
                                              aps[m][arows, 0:256])
                # normalize this 256-col query block: 1/sums broadcast to the
                # two 64-row head blocks via K=1 outer products, multiplied
                # straight out of PSUM.
                for t, h0, h1 in ((stk0, 0, 1), (stk1, 2, 3)):
                    rb_ps = pnorm.tile([128, 256], F32, tag="rbp", name="sinv_ps")
                    nc.tensor.matmul(rb_ps[:], ones_lo[:],
                                     sinv_sb[0:1, h0 * S + it * 256:h0 * S + it * 256 + 256],
                                     start=True, stop=False)
                    nc.tensor.matmul(rb_ps[:], ones_hi[:],
                                     sinv_sb[0:1, h1 * S + it * 256:h1 * S + it * 256 + 256],
                                     start=False, stop=True)
                    nc.vector.tensor_tensor(t[:, isl], t[:, isl], rb_ps[:],
                                            AluOpType.mult)
                # wo contribution of this query block
                ito = it - 2 * st
                for dmm in range(16):
                    dsl = slice(dmm * 128, (dmm + 1) * 128)
                    wops = pa.tile([128, 256], F32, tag=wo_tags[dmm % 4], name="wops")
                    nc.tensor.matmul(wops[:], wo0[:, dsl], stk0[:, isl],
                                     start=True, stop=False)
                    nc.tensor.matmul(wops[:], wo1[:, dsl], stk1[:, isl],
                                     start=False, stop=True)
                    osl = slice((dmm % 4) * 512 + ito * 256,
                                (dmm % 4) * 512 + ito * 256 + 256)
                    xsl = xtb[:, dmm * S + it * 256:dmm * S + it * 256 + 256]
                    nc.vector.scalar_tensor_tensor(
                        stgs[dmm // 4][:, osl], xsl, 1.0 / NCORES, wops[:],
                        AluOpType.mult, AluOpType.add)
            for j in range(4):
                nc.scalar.dma_start(
                    bass.AP(tensor=cc_in.tensor,
                            offset=cc_in.offset + j * 4 * 128 * HS,
                            ap=[[HS, 128], [128 * HS, 4], [1, HS]]),
                    stgs[j][:])

        def gu_half(pm, l, prod, st):
            for fmh in range(2):
                fms = [4 * fmh + j for j in range(4)]
                for phase, w_in in (("g", wg_in), ("u", wu_in)):
                    ps = {}
                    for fm in fms:
                        ps[fm] = pm.tile([128, HS], F32, tag=f"m{fm % 4}",
                                         name=f"m{phase}{fm}")
                    for g in range(8):
                        wt = wbig.tile([128, 1024], F16, tag="wgu", name="wgu", bufs=4)
                        nc.scalar.dma_start(
                            wt[:],
                            bass.AP(tensor=w_in.tensor,
                                    offset=(w_in.offset + l * DM * FFS + g * 2 * 128 * FFS
                                            + fmh * 512),
                                    ap=[[FFS, 128], [128 * FFS, 2], [1, 512]]))
                        for i in range(2):
                            k = g * 2 + i
                            for fm in fms:
                                wsl = wt[:, i * 512 + (fm % 4) * 128:
                                         i * 512 + (fm % 4 + 1) * 128]
                                nc.tensor.matmul(ps[fm][:], wsl, XBN(k, st),
                                                 start=(k == 0), stop=(k == KT - 1))
                    if phase == "g":
                        sil = {}
                        for fm in fms:
                            t = actp.tile([128, HS], F16, tag=f"sil{fm % 4}",
                                          name=f"sil{fm}")
                            nc.scalar.activation(t[:], ps[fm][:], AF.Silu)
                            sil[fm] = t
                    else:
                        for fm in fms:
                            t = actp.tile([128, HS], F16, tag=f"prod{fm}",
                                          name=f"prod{fm}")
                            nc.vector.tensor_tensor(t[:], sil[fm][:], ps[fm][:],
                                                    AluOpType.mult)
                            prod[fm] = t

        def down_half(pm, l, prod, cc_in, st, last):
            stsl = slice(st * HS, (st + 1) * HS)
            for dmg in range(4):
                dps = {}
                for d in range(4):
                    dps[d] = pm.tile([128, HS], F32, tag=f"m{d}", name=f"md{d}")
                for gg in range(2):
                    wdt = wbig.tile([128, 2048], F16, tag="wdt", name="wdt", bufs=3)
                    nc.scalar.dma_start(
                        wdt[:],
                        bass.AP(tensor=wd_in.tensor,
                                offset=(wd_in.offset + l * FFS * DM + gg * 4 * 128 * DM
                                        + dmg * 512),
                                ap=[[DM, 128], [128 * DM, 4], [1, 512]]))
                    for i2 in range(4):
                        fk = gg * 4 + i2
                        for d in range(4):
                            wsl = wdt[:, i2 * 512 + d * 128:i2 * 512 + (d + 1) * 128]
                            nc.tensor.matmul(dps[d][:], wsl, prod[fk][:],
                                             start=(fk == 0), stop=(fk == FMT - 1))
                for dp in range(2):
                    stg = stage.tile([128, 1024], F16, tag="dstg")
                    for i in range(2):
                        d = dp * 2 + i
                        kk = dmg * 4 + d
                        osl = slice(i * HS, (i + 1) * HS)
                        nc.vector.scalar_tensor_tensor(
                            stg[:, osl], XT(kk, st), 1.0 / NCORES, dps[d][:],
                            AluOpType.mult, AluOpType.add)
                    dst = y_out if last else cc_in
                    if last:
                        dstap = bass.AP(
                            tensor=dst.tensor,
                            offset=dst.offset + (dmg * 4 + dp * 2) * 128 * S + st * HS,
                            ap=[[S, 128], [128 * S, 2], [1, HS]])
                    else:
                        dstap = bass.AP(
                            tensor=dst.tensor,
                            offset=dst.offset + (dmg * 4 + dp * 2) * 128 * HS,
                            ap=[[HS, 128], [128 * HS, 2], [1, HS]])
                    nc.scalar.dma_start(dstap, stg[:])

        def allreduce(cc_in, cc_out):
            nc.gpsimd.collective_compute(
                "AllReduce", AluOpType.add,
                replica_groups=[list(range(NCORES))],
                ins=[cc_in[:].opt()], outs=[cc_out[:].opt()])

        def cc_pair(tagbase, st):
            cc_in = dram.tile([DM, HS], F16, tag=f"{tagbase}i{st}", name="cc_in")
            if skip_cc:
                return cc_in, cc_in
            cc_out = dram.tile([DM, HS], F16, tag=f"{tagbase}o{st}", name="cc_out",
                               addr_space="Shared")
            if cc_nowait:
                return cc_in, cc_in, cc_out
            return cc_in, cc_out

        # ---------- main program ----------
        for _ in range(reps):
            load_x()
            # mlp-norm carried across the layer boundary
            pend_mlp_cc = [None, None]
            for l in range(L):
                ctx_a = ExitStack()
                pa = ctx_a.enter_context(tc.tile_pool(name="pa", bufs=1, space="PSUM"))
                wqa = wqp.tile([128, KT * 384], F16, tag="wqa", name="wqa")
                for g in range(4):
                    nc.scalar.dma_start(
                        wqa[:, g * 4 * 384:(g + 1) * 4 * 384],
                        bass.AP(tensor=wqkv_in.tensor,
                                offset=wqkv_in.offset + l * DM * 384 + g * 4 * 128 * 384,
                                ap=[[384, 128], [128 * 384, 4], [1, 384]]))
                wo0 = wob.tile([128, DM], F16, tag="wo0")
                nc.sync.dma_start(wo0[:], wo_in[l, 0:128, :])
                wo1 = wob.tile([128, DM], F16, tag="wo1")
                nc.sync.dma_start(wo1[:], wo_in[l, 128:256, :])

                q01 = attn_sb.tile([128, S], F16, tag="q01")
                q23 = attn_sb.tile([128, S], F16, tag="q23")
                kv2 = attn_sb.tile([128, S], F16, tag="kv2")
                kt2 = attn_sb.tile([128, S], F16, tag="kt2")
                stk0 = stkp.tile([128, S], F16, tag="stk0")
                stk1 = stkp.tile([128, S], F16, tag="stk1")
                sinv_sb = stkp.tile([1, 4 * S], F16, tag="sinv_sb")
                va = []
                cc_a = [cc_pair("a", st) for st in range(2)]

                for st in range(2):
                    if pend_mlp_cc[st] is not None:
                        residual_and_norm_half(pa, pend_mlp_cc[st], st)
                        pend_mlp_cc[st] = None
                    qkv_rope_half(pa, wqa, q01, q23, kv2, kt2, st)
                    vtp(pa, kv2, va, 4 * st)
                    vtp(pa, kv2, va, 4 * st + 1)
                    attn_half(pa, q01, q23, kt2, kv2, va, stk0, stk1, sinv_sb,
                              wo0, wo1, cc_a[st][0], st)
                    if not skip_cc:
                        allreduce(cc_a[st][0], cc_a[st][-1])

                if debug_stage == f"stk{l}":
                    for ti, t in enumerate((stk0, stk1)):
                        nc.sync.dma_start(y_out[ti * 128:(ti + 1) * 128, :], t[:])
                    ctx_a.close()
                    break
                ctx_a.close()

                last = (l == L - 1 and debug_stage is None)
                ctx_m = ExitStack()
                pm = ctx_m.enter_context(tc.tile_pool(name="pm", bufs=1, space="PSUM"))
                cc_m = [None if last else cc_pair("m", st) for st in range(2)]
                for st in range(2):
                    residual_and_norm_half(pm, cc_a[st][1], st)
                    prod = {}
                    gu_half(pm, l, prod, st)
                    down_half(pm, l, prod, None if last else cc_m[st][0], st, last)
                    if not last and not skip_cc:
                        allreduce(cc_m[st][0], cc_m[st][-1])
                    if not last:
                        pend_mlp_cc[st] = cc_m[st][1]
                ctx_m.close()
                if debug_stage == f"attn{l}" or debug_stage == f"mlp{l}":
                    for k in range(KT):
                        nc.sync.dma_start(y_out[k * 128:(k + 1) * 128, :], XT(k))
                    break

    nc.compile()
    return nc


def make_inputs(input_ids, embed, wq, wk, wv, wo, wgate, wup, wdown, ln1, ln2):
    """host-side prep: embedding gather, layer0-norm, shard + fold gains."""
    f32 = np.float32
    f16 = np.float16
    x = np.asarray(embed, f32)[np.asarray(input_ids)[0]]      # (S, DM)
    xt = np.ascontiguousarray(x.T)                            # (DM, S)
    r0 = 1.0 / np.sqrt(np.mean(xt * xt, axis=0) + EPS)        # (S,)
    xbn0 = (xt * r0[None, :]).astype(f16)

    inv_freq = 1.0 / (THETA ** (np.arange(0, D, 2, dtype=f32) / D))
    freqs = np.arange(S, dtype=f32)[:, None] * inv_freq[None, :]    # (S, 32)
    emb = np.concatenate([freqs, freqs], axis=1)                    # (S, D)
    cosT = np.cos(emb).T.astype(f32)                                # (D, S)
    sinT = np.sin(emb).T.astype(f32)
    sinT_signed = sinT.copy()
    sinT_signed[: D // 2] *= -1.0
    cos_rep = np.concatenate([cosT, cosT], axis=0).astype(f16)      # (128, S)
    sin_rep = np.concatenate([sinT_signed, sinT_signed], axis=0).astype(f16)

    step = np.zeros((2, 128, 256), f16)
    jj = np.arange(128)[:, None]
    ii = np.arange(256)[None, :]
    for o in range(2):
        step[o] = (128 * o + jj > ii).astype(f16)
    negbigI = (NEGBIG * np.eye(128, dtype=f32)).astype(f16)
    ident = np.concatenate([np.eye(64), np.eye(64)], axis=0).astype(f16)
    cpack = np.concatenate([step[0], step[1], negbigI, ident], axis=1).astype(f16)

    scale_q = 1.0 / math.sqrt(D)
    in_maps = []
    for c in range(NCORES):
        wqkv = np.empty((L, DM, 384), f32)
        wo_c = np.empty((L, QC, DM), f32)
        wg_c = np.empty((L, DM, FFS), f32)
        wu_c = np.empty((L, DM, FFS), f32)
        wd_c = np.empty((L, FFS, DM), f32)
        for l in range(L):
            g1 = np.asarray(ln1[l], f32)[:, None]
            g2 = np.asarray(ln2[l], f32)[:, None]
            wqkv[l, :, :QC] = np.asarray(wq[l], f32)[:, c * QC:(c + 1) * QC] * g1 * scale_q
            wqkv[l, :, QC:QC + D] = np.asarray(wk[l], f32)[:, c * D:(c + 1) * D] * g1
            wqkv[l, :, QC + D:] = np.asarray(wv[l], f32)[:, c * D:(c + 1) * D] * g1
            wo_c[l] = np.asarray(wo[l], f32)[c * QC:(c + 1) * QC, :]
            wg_c[l] = np.asarray(wgate[l], f32)[:, c * FFS:(c + 1) * FFS] * g2
            wu_c[l] = np.asarray(wup[l], f32)[:, c * FFS:(c + 1) * FFS] * g2
            wd_c[l] = np.asarray(wdown[l], f32)[c * FFS:(c + 1) * FFS, :]
        in_maps.append({
            "xt16": xt.astype(f16), "xbn0": xbn0, "wqkv": wqkv.astype(f16),
            "wo": wo_c.astype(f16), "wg": wg_c.astype(f16), "wu": wu_c.astype(f16),
            "wd": wd_c.astype(f16), "cosr": cos_rep, "sinr": sin_rep,
            "cpack": cpack,
        })
    return in_maps


_NC_CACHE = {}


def kernel(**inputs) -> np.ndarray:
    if 1 not in _NC_CACHE:
        _NC_CACHE[1] = build(reps=1)
    nc = _NC_CACHE[1]
    in_maps = make_inputs(**inputs)
    res = run_bass_kernel_spmd(nc, in_maps, list(range(NCORES)))
    y = np.zeros((DM, S), np.float64)
    for c in range(NCORES):
        y += res.results[c]["y"].astype(np.float64)
    return np.ascontiguousarray(y.T.astype(np.float32)).reshape(B, S, DM)


# revision 37
# speedup vs baseline: 1.3220x; 1.0363x over previous
"""Llama trunk (2 layers, before final norm) on 8 trn2 cores.

Sharding: Megatron tensor-parallel over 8 cores.
  - attention: 4 q-heads + 1 kv-head per core (GQA group stays local)
  - MLP: 1024 of 8192 intermediate dims per core
  - residual stream xt kept in fp16, transposed [DM(part), S(free)]

v2: token-half software pipeline. All per-token phases (qkv, rope, wo,
norm, MLP) are processed in two 512-column halves T0/T1, and the three
on-device AllReduces are split into six half-payload AllReduces, each
overlapped with the other half's compute (collectives run on dedicated
SDMA/CCE silicon, so the PE keeps streaming through them). Attention is
causal, so T0 queries only need T0 keys and the attention block also
pipelines by half. Other changes vs v1:
  - softmax 1/sum is broadcast to the 64-row head blocks with two K=1
    outer-product matmuls into PSUM (ones_lo/ones_hi stationaries)
    instead of a DRAM round-trip.
  - qkv is two passes (q-pass: 2 psums, kv-pass: 1) so attention fits
    the 8 PSUM banks together with wo / norm psums; weights stay in
    SBUF across both passes and both halves.
  - odd-parity softmax key-sums go to one packed PSUM bank (partition
    0 / 32 via tile_position) instead of two.
"""
import math
from contextlib import ExitStack

import numpy as np

import concourse.bass as bass
import concourse.tile as tile
from concourse import bacc, mybir
from concourse.alu_op_type import AluOpType
from concourse.bass_utils import run_bass_kernel_spmd

L, H, KVH, D = 2, 32, 8, 64
DM, FF = 2048, 8192
B, S = 1, 1024
EPS, THETA = 1e-5, 10000.0
NCORES = 8
QH = H // NCORES            # 4 q heads per core
QC = QH * D                 # 256 q cols per core
FFS = FF // NCORES          # 1024 ff dims per core
KT = DM // 128              # 16 contraction tiles over DM
FMT = FFS // 128            # 8 f tiles
HS = S // 2                 # 512 token half
NEGBIG = -30000.0

F32 = mybir.dt.float32
F16 = mybir.dt.float16
AF = mybir.ActivationFunctionType


def build(reps=1, debug_stage=None, skip_cc=False, cc_nowait=False,
          ar_mode="half"):
    nc = bacc.Bacc(None, target_bir_lowering=False, debug=False, num_devices=NCORES)
    xt_in = nc.dram_tensor("xt16", [DM, S], F16, kind="ExternalInput").ap()
    xbn0_in = nc.dram_tensor("xbn0", [DM, S], F16, kind="ExternalInput").ap()
    wqkv_in = nc.dram_tensor("wqkv", [L, DM, 384], F16, kind="ExternalInput").ap()
    wo_in = nc.dram_tensor("wo", [L, QC, DM], F16, kind="ExternalInput").ap()
    wg_in = nc.dram_tensor("wg", [L, DM, FFS], F16, kind="ExternalInput").ap()
    wu_in = nc.dram_tensor("wu", [L, DM, FFS], F16, kind="ExternalInput").ap()
    wd_in = nc.dram_tensor("wd", [L, FFS, DM], F16, kind="ExternalInput").ap()
    cos_in = nc.dram_tensor("cosr", [128, S], F16, kind="ExternalInput").ap()
    sin_in = nc.dram_tensor("sinr", [128, S], F16, kind="ExternalInput").ap()
    cpack_in = nc.dram_tensor("cpack", [128, 704], F16, kind="ExternalInput").ap()
    y_out = nc.dram_tensor("y", [DM, S], F16, kind="ExternalOutput").ap()

    with tile.TileContext(nc) as tc, ExitStack() as ctx, \
            nc.allow_low_precision(reason="deliberate fp16 pipeline, tol 2e-2"):
        const = ctx.enter_context(tc.tile_pool(name="const", bufs=1))
        xtp = ctx.enter_context(tc.tile_pool(name="xtp", bufs=1))
        wqp = ctx.enter_context(tc.tile_pool(name="wqp", bufs=1))
        wob = ctx.enter_context(tc.tile_pool(name="wob", bufs=1))
        wbig = ctx.enter_context(tc.tile_pool(name="wbig", bufs=2))
        sq = ctx.enter_context(tc.tile_pool(name="sq", bufs=2))
        rp = ctx.enter_context(tc.tile_pool(name="rp", bufs=2))
        attn_sb = ctx.enter_context(tc.tile_pool(name="attn_sb", bufs=1))
        ropet = ctx.enter_context(tc.tile_pool(name="ropet", bufs=2))
        vap = ctx.enter_context(tc.tile_pool(name="vap", bufs=1))
        expp = ctx.enter_context(tc.tile_pool(name="expp", bufs=2))
        stkp = ctx.enter_context(tc.tile_pool(name="stkp", bufs=1))
        stage = ctx.enter_context(tc.tile_pool(name="stage", bufs=2))
        arp = ctx.enter_context(tc.tile_pool(name="arp", bufs=2))
        actp = ctx.enter_context(tc.tile_pool(name="actp", bufs=1))

        dram = ctx.enter_context(tc.tile_pool(name="dram", bufs=2, space="DRAM"))
        # persistent psum: norm sumsq / odd softmax sums (1 bank) + row
        # broadcasts (1 bank)
        pnorm = ctx.enter_context(tc.tile_pool(name="pnorm", bufs=1, space="PSUM"))

        # ---- persistent constants ----
        onesb = const.tile([128, 1], F16)
        nc.vector.memset(onesb[:], 1.0)
        onesr = const.tile([1, 128], F16)
        nc.vector.memset(onesr[:], 1.0)
        ones_lo = const.tile([1, 128], F16)
        nc.vector.memset(ones_lo[0:1, 0:64], 1.0)
        nc.vector.memset(ones_lo[0:1, 64:128], 0.0)
        ones_hi = const.tile([1, 128], F16)
        nc.vector.memset(ones_hi[0:1, 0:64], 0.0)
        nc.vector.memset(ones_hi[0:1, 64:128], 1.0)
        # small consts in ONE packed DMA, first on sync (scav's mask
        # matmuls can get scheduled early and head-block the PE queue on
        # these); big cos/sin tables on the gpsimd DGE queue
        cpk = const.tile([128, 704], F16)
        nc.sync.dma_start(cpk[:], cpack_in[:])
        step0 = cpk[:, 0:256]
        step1 = cpk[:, 256:512]
        nbi = cpk[:, 512:640]
        ident = cpk[:, 640:704]
        cosr = const.tile([128, S], F16)
        nc.gpsimd.dma_start(cosr[:], cos_in[:])
        sinr = const.tile([128, S], F16)
        nc.gpsimd.dma_start(sinr[:], sin_in[:])
        epsb = const.tile([1, 1], F32)
        nc.vector.memset(epsb[:], EPS)

        # residual stream + normalized copy, fp16, resident (big tiles)
        xtb = xtp.tile([128, KT * S], F16, tag="xtb", name="xtb")
        xbnb = xtp.tile([128, KT * S], F16, tag="xbnb", name="xbnb")

        def XT(k, st=None):
            if st is None:
                return xtb[:, k * S:(k + 1) * S]
            return xtb[:, k * S + st * HS:k * S + (st + 1) * HS]

        def XBN(k, st=None):
            if st is None:
                return xbnb[:, k * S:(k + 1) * S]
            return xbnb[:, k * S + st * HS:k * S + (st + 1) * HS]

        xtb3 = xtb.rearrange("p (k s) -> p k s", k=KT)
        xbnb3 = xbnb.rearrange("p (k s) -> p k s", k=KT)

        def load_x():
            # xbn0 in 4 contiguous 4-k-tile chunks so qkv(T0) starts after
            # the first chunk (subtile deps). xt goes on the gpsimd queue.
            for c in range(4):
                nc.sync.dma_start(
                    xbnb[:, 4 * c * S:(4 * c + 4) * S],
                    bass.AP(tensor=xbn0_in.tensor,
                            offset=xbn0_in.offset + 4 * c * 128 * S,
                            ap=[[S, 128], [128 * S, 4], [1, S]]))
            for st in range(2):
                nc.gpsimd.dma_start(
                    xtb3[:, :, st * HS:(st + 1) * HS],
                    bass.AP(tensor=xt_in.tensor, offset=xt_in.offset + st * HS,
                            ap=[[S, 128], [128 * S, KT], [1, HS]]))

        # ---------- per layer-token-half pieces ----------

        def residual_and_norm_half(pa, cc_out, st):
            """cc_out already holds x_new = residual + block output (the x/8
            fold happens in the stage evacuations, so the AllReduce sums it).
            Here: sumsq stats; r; xbn(st); lazy copy of x_new into xtb."""
            ssum = pnorm.tile([1, HS], F32, tag="ssum", name="ssum")
            queues = (nc.sync, nc.scalar, nc.gpsimd)
            ar_ws = []
            for q in range(8):
                ar_w = arp.tile([128, 2 * HS], F16, tag=f"ar{q}", bufs=1, name="ar")
                queues[q % 3].dma_start(ar_w[:], ccap(cc_out, q * 2, 2, st))
                ar_ws.append(ar_w)
            def ARW(k):
                return ar_ws[k // 2][:, (k % 2) * HS:(k % 2 + 1) * HS]
            for k in range(KT):
                xsq = sq.tile([128, HS], F16, tag="xsq")
                nc.vector.tensor_tensor(xsq[:], ARW(k), ARW(k), AluOpType.mult)
                nc.tensor.matmul(ssum[:], onesb[:], xsq[:],
                                 start=(k == 0), stop=(k == KT - 1))
            rs = rp.tile([1, HS], F32, tag="rs")
            nc.scalar.activation(rs[:], ssum[:], AF.Sqrt, bias=epsb[:], scale=1.0 / DM)
            rrf = rp.tile([1, HS], F32, tag="rrf")
            nc.vector.reciprocal(rrf[:], rs[:])
            rr16 = rp.tile([1, HS], F16, tag="rr16")
            nc.vector.tensor_copy(rr16[:], rrf[:])
            rb_ps = pnorm.tile([128, HS], F32, tag="rbp", name="rb_ps")
            nc.tensor.matmul(rb_ps[:], onesr[:], rr16[:], start=True, stop=True)
            rb = rp.tile([128, HS], F16, tag="rb")
            nc.vector.tensor_copy(rb[:], rb_ps[:])
            for k in range(KT):
                nc.vector.tensor_tensor(XBN(k, st), ARW(k), rb[:], AluOpType.mult)
            # x_new -> xtb off the critical path (needed by the next block's
            # stage fold, ~40us later)
            for k in range(KT):
                nc.vector.tensor_copy(XT(k, st), ARW(k))

        def rope_one(t, nrows, out, outrows, stsl):
            rot = ropet.tile([128, HS], F16, tag="rot")
            for h0 in range(0, nrows, 64):
                nc.sync.dma_start(rot[h0:h0 + 32, :], t[h0 + 32:h0 + 64, stsl])
                nc.sync.dma_start(rot[h0 + 32:h0 + 64, :], t[h0:h0 + 32, stsl])
            t1 = ropet.tile([128, HS], F16, tag="t1")
            nc.vector.tensor_tensor(t1[0:nrows, :], t[0:nrows, stsl],
                                    cosr[0:nrows, stsl], AluOpType.mult)
            t2 = ropet.tile([128, HS], F16, tag="t2")
            nc.vector.tensor_tensor(t2[0:nrows, :], rot[0:nrows, :],
                                    sinr[0:nrows, stsl], AluOpType.mult)
            nc.vector.tensor_add(out[outrows, stsl], t1[0:nrows, :], t2[0:nrows, :])

        def qkv_rope_half(pa, wqa, q01, q23, kv2, kt2, st):
            """q-pass, rope(q) emitted before kv-pass so it overlaps on DVE."""
            stsl = slice(st * HS, (st + 1) * HS)
            pq0 = pa.tile([128, HS], F32, tag="pq0", name="pq0")
            pq1 = pa.tile([128, HS], F32, tag="pq1", name="pq1")
            for k in range(KT):
                w0 = k * 384
                st_, sp_ = (k == 0), (k == KT - 1)
                nc.tensor.matmul(pq0[:], wqa[:, w0:w0 + 128], XBN(k, st),
                                 start=st_, stop=sp_)
                nc.tensor.matmul(pq1[:], wqa[:, w0 + 128:w0 + 256], XBN(k, st),
                                 start=st_, stop=sp_)
            nc.vector.tensor_copy(q01[:, stsl], pq0[:])
            nc.vector.tensor_copy(q23[:, stsl], pq1[:])
            rope_one(q01, 128, q01, slice(0, 128), stsl)
            rope_one(q23, 128, q23, slice(0, 128), stsl)
            # kv-pass: one psum; its matmuls overlap the q-rope DVE work
            pkv = pa.tile([128, HS], F32, tag="sps0", name="pkv")
            for k in range(KT):
                w0 = k * 384
                nc.tensor.matmul(pkv[:], wqa[:, w0 + 256:w0 + 384], XBN(k, st),
                                 start=(k == 0), stop=(k == KT - 1))
            # k rows first so rope-k starts while v evacuates
            nc.scalar.copy(kv2[0:64, stsl], pkv[0:64, :])
            nc.scalar.copy(kv2[64:128, stsl], pkv[64:128, :])
            rope_one(kv2, 64, kt2, slice(0, 64), stsl)
            nc.sync.dma_start(kt2[64:128, stsl], kt2[0:64, stsl])

        def vtp(pa, kv2, va, sj):
            tp = pa.tile([128, 64], F16, tag=f"aps{sj % 2}", name="tp")
            nc.tensor.transpose(tp[:], kv2[64:128, sj * 128:(sj + 1) * 128],
                                ident[64:128, :])
            v = vap.tile([128, 65], F16, tag=f"va{sj}", name=f"va{sj}")
            nc.vector.tensor_copy(v[:, 0:64], tp[:])
            nc.vector.memset(v[:, 64:65], 1.0)
            va.append(v)

        def attn_half(pa, q01, q23, kt2, kv2, va, stk0, stk1, sinv_sb,
                      wo0, wo1, cc_in, st):
            """scores+exp+av, per-block normalize, and per-block wo for the
            query blocks of half st (it in 2st..2st+1).

            it-outer so both parities of a query block finish together; the
            block is then normalized and its wo contribution computed while
            the next block's scores run. v-transposes for the later key
            blocks are emitted mid-stream to fill the softmax warmup.
            """
            wo_tags = ("pq0", "pq1", "sps0", "sps1")
            stgs = [stage.tile([128, 2048], F16, tag=f"wostg{j % 2}",
                               name=f"stg{j}") for j in range(4)]
            jp_ctr = 0
            for it in range(2 * st, 2 * st + 2):
                for parity in range(2):
                    rows = slice(64 * parity, 64 * parity + 64)
                    odd = parity == 1
                    isl = slice(it * 256, (it + 1) * 256)
                    aps = [pa.tile([128, 512], F32, tag=f"aps{m}", name=f"aps{m}")
                           for m in range(2)]
                    for jp in range(it + 1):
                        diag = jp == it
                        tg = ("pq0", "pq1") if jp_ctr % 2 else ("sps0", "sps1")
                        jp_ctr += 1
                        sps = [pa.tile([128, 512], F32, tag=tg[m], name=f"sps{m}")
                               for m in range(2)]
                        for half in range(2):
                            j = 2 * jp + half
                            ssl = slice(half * 256, half * 256 + 256)
                            if diag:
                                for m in range(2):
                                    nc.tensor.matmul(sps[m][:, ssl], nbi[:],
                                                     (step0, step1)[half][:],
                                                     start=True, stop=False)
                            for m, qt in enumerate((q01, q23)):
                                nc.tensor.matmul(sps[m][:, ssl],
                                                 kt2[rows, j * 128:(j + 1) * 128],
                                                 qt[rows, isl],
                                                 start=not diag, stop=True)
                        es = []
                        for m in range(2):
                            e = expp.tile([128, 512], F16, tag=f"e{m}", name=f"e{m}")
                            nc.scalar.activation(e[:], sps[m][:], AF.Exp)
                            es.append(e)
                        if it == 2 * st and parity == 0 and jp == 0:
                            # transposes for the later key blocks fill the
                            # scores->exp->av pipeline warmup with PE work
                            vtp(pa, kv2, va, 4 * st + 2)
                            vtp(pa, kv2, va, 4 * st + 3)
                        first, last = (jp == 0), (jp == it)
                        for half in range(2):
                            esl = slice(half * 256, half * 256 + 256)
                            vsl = slice(0, 64) if odd else slice(0, 65)
                            orows = slice(64, 128) if odd else slice(0, 65)
                            for m in range(2):
                                # odd parity: av (rows 64:128) and key-sums
                                # (row 0) share a bank as two accumulation
                                # groups on disjoint partitions — PSUM
                                # pending-zero tracking is per-partition.
                                nc.tensor.matmul(aps[m][orows, 0:256],
                                                 va[2 * jp + half][:, vsl], es[m][:, esl],
                                                 start=(first and half == 0),
                                                 stop=(last and half == 1))
                            if odd:
                                for m in range(2):
                                    nc.tensor.matmul(
                                        aps[m][0:1, 0:256],
                                        onesb[:], es[m][:, esl],
                                        start=(first and half == 0),
                                        stop=(last and half == 1))
                    arows = slice(64, 128) if odd else slice(0, 64)
                    heads = (parity, parity + 2)
                    for m in range(2):
                        h = heads[m]
                        srow = (aps[m][0:1, 0:256] if odd
                                else aps[m][64:65, 0:256])
                        nc.vector.reciprocal(
                            sinv_sb[0:1, h * S + it * 256:h * S + it * 256 + 256], srow)
                        nc.vector.tensor_copy((stk0, stk1)[m][arows, isl],
                                              aps[m][arows, 0:256])
                # normalize this 256-col query block: 1/sums broadcast to the
                # two 64-row head blocks via K=1 outer products, multiplied
                # straight out of PSUM.
                for t, h0, h1 in ((stk0, 0, 1), (stk1, 2, 3)):
                    rb_ps = pnorm.tile([128, 256], F32, tag="rbp", name="sinv_ps")
                    nc.tensor.matmul(rb_ps[:], ones_lo[:],
                                     sinv_sb[0:1, h0 * S + it * 256:h0 * S + it * 256 + 256],
                                     start=True, stop=False)
                    nc.tensor.matmul(rb_ps[:], ones_hi[:],
                                     sinv_sb[0:1, h1 * S + it * 256:h1 * S + it * 256 + 256],
                                     start=False, stop=True)
                    nc.vector.tensor_tensor(t[:, isl], t[:, isl], rb_ps[:],
                                            AluOpType.mult)
                # wo contribution of this query block
                ito = it - 2 * st
                for dmm in range(16):
                    dsl = slice(dmm * 128, (dmm + 1) * 128)
                    wops = pa.tile([128, 256], F32, tag=wo_tags[dmm % 4], name="wops")
                    nc.tensor.matmul(wops[:], wo0[:, dsl], stk0[:, isl],
                                     start=True, stop=False)
                    nc.tensor.matmul(wops[:], wo1[:, dsl], stk1[:, isl],
                                     start=False, stop=True)
                    osl = slice((dmm % 4) * 512 + ito * 256,
                                (dmm % 4) * 512 + ito * 256 + 256)
                    xsl = xtb[:, dmm * S + it * 256:dmm * S + it * 256 + 256]
                    nc.vector.scalar_tensor_tensor(
                        stgs[dmm // 4][:, osl], xsl, 1.0 / NCORES, wops[:],
                        AluOpType.mult, AluOpType.add)
            for j in range(4):
                nc.scalar.dma_start(ccap(cc_in, j * 4, 4, st), stgs[j][:])

        def gu_half(pm, l, prod, st):
            for fmh in range(2):
                fms = [4 * fmh + j for j in range(4)]
                for phase, w_in in (("g", wg_in), ("u", wu_in)):
                    ps = {}
                    for fm in fms:
                        ps[fm] = pm.tile([128, HS], F32, tag=f"m{fm % 4}",
                                         name=f"m{phase}{fm}")
                    for g in range(8):
                        wt = wbig.tile([128, 1024], F16, tag="wgu", name="wgu", bufs=4)
                        nc.scalar.dma_start(
                            wt[:],
                            bass.AP(tensor=w_in.tensor,
                                    offset=(w_in.offset + l * DM * FFS + g * 2 * 128 * FFS
                                            + fmh * 512),
                                    ap=[[FFS, 128], [128 * FFS, 2], [1, 512]]))
                        for i in range(2):
                            k = g * 2 + i
                            for fm in fms:
                                wsl = wt[:, i * 512 + (fm % 4) * 128:
                                         i * 512 + (fm % 4 + 1) * 128]
                                nc.tensor.matmul(ps[fm][:], wsl, XBN(k, st),
                                                 start=(k == 0), stop=(k == KT - 1))
                    if phase == "g":
                        sil = {}
                        for fm in fms:
                            t = actp.tile([128, HS], F16, tag=f"sil{fm % 4}",
                                          name=f"sil{fm}")
                            nc.scalar.activation(t[:], ps[fm][:], AF.Silu)
                            sil[fm] = t
                    else:
                        for fm in fms:
                            t = actp.tile([128, HS], F16, tag=f"prod{fm}",
                                          name=f"prod{fm}")
                            nc.vector.tensor_tensor(t[:], sil[fm][:], ps[fm][:],
                                                    AluOpType.mult)
                            prod[fm] = t

        def down_half(pm, l, prod, cc_in, st, last):
            stsl = slice(st * HS, (st + 1) * HS)
            for dmg in range(4):
                dps = {}
                for d in range(4):
                    dps[d] = pm.tile([128, HS], F32, tag=f"m{d}", name=f"md{d}")
                for gg in range(2):
                    wdt = wbig.tile([128, 2048], F16, tag="wdt", name="wdt", bufs=3)
                    nc.scalar.dma_start(
                        wdt[:],
                        bass.AP(tensor=wd_in.tensor,
                                offset=(wd_in.offset + l * FFS * DM + gg * 4 * 128 * DM
                                        + dmg * 512),
                                ap=[[DM, 128], [128 * DM, 4], [1, 512]]))
                    for i2 in range(4):
                        fk = gg * 4 + i2
                        for d in range(4):
                            wsl = wdt[:, i2 * 512 + d * 128:i2 * 512 + (d + 1) * 128]
                            nc.tensor.matmul(dps[d][:], wsl, prod[fk][:],
                                             start=(fk == 0), stop=(fk == FMT - 1))
                for dp in range(2):
                    stg = stage.tile([128, 1024], F16, tag="dstg")
                    for i in range(2):
                        d = dp * 2 + i
                        kk = dmg * 4 + d
                        osl = slice(i * HS, (i + 1) * HS)
                        nc.vector.scalar_tensor_tensor(
                            stg[:, osl], XT(kk, st), 1.0 / NCORES, dps[d][:],
                            AluOpType.mult, AluOpType.add)
                    if last:
                        dstap = bass.AP(
                            tensor=y_out.tensor,
                            offset=y_out.offset + (dmg * 4 + dp * 2) * 128 * S + st * HS,
                            ap=[[S, 128], [128 * S, 2], [1, HS]])
                    else:
                        dstap = ccap(cc_in, dmg * 4 + dp * 2, 2, st)
                    nc.scalar.dma_start(dstap, stg[:])

        def allreduce(cc_in, cc_out):
            nc.gpsimd.collective_compute(
                "AllReduce", AluOpType.add,
                replica_groups=[list(range(NCORES))],
                ins=[cc_in[:].opt()], outs=[cc_out[:].opt()])

        CCW = HS if ar_mode == "half" else S

        def ccap(cc, tile0, ntiles, st):
            off = cc.offset + tile0 * 128 * CCW + (st * HS if CCW == S else 0)
            return bass.AP(tensor=cc.tensor, offset=off,
                           ap=[[CCW, 128], [128 * CCW, ntiles], [1, HS]])

        def cc_pair(tagbase, st):
            cc_in = dram.tile([DM, CCW], F16, tag=f"{tagbase}i{st}", name="cc_in")
            if skip_cc:
                return cc_in, cc_in
            cc_out = dram.tile([DM, CCW], F16, tag=f"{tagbase}o{st}", name="cc_out",
                               addr_space="Shared")
            if cc_nowait:
                return cc_in, cc_in, cc_out
            return cc_in, cc_out

        # ---------- main program ----------
        for _ in range(reps):
            load_x()
            # mlp-norm carried across the layer boundary
            pend_mlp_cc = [None, None]
            for l in range(L):
                ctx_a = ExitStack()
                pa = ctx_a.enter_context(tc.tile_pool(name="pa", bufs=1, space="PSUM"))
                wqa = wqp.tile([128, KT * 384], F16, tag="wqa", name="wqa")
                for g in range(4):
                    nc.scalar.dma_start(
                        wqa[:, g * 4 * 384:(g + 1) * 4 * 384],
                        bass.AP(tensor=wqkv_in.tensor,
                                offset=wqkv_in.offset + l * DM * 384 + g * 4 * 128 * 384,
                                ap=[[384, 128], [128 * 384, 4], [1, 384]]))
                wo0 = wob.tile([128, DM], F16, tag="wo0")
                nc.sync.dma_start(wo0[:], wo_in[l, 0:128, :])
                wo1 = wob.tile([128, DM], F16, tag="wo1")
                nc.sync.dma_start(wo1[:], wo_in[l, 128:256, :])

                q01 = attn_sb.tile([128, S], F16, tag="q01")
                q23 = attn_sb.tile([128, S], F16, tag="q23")
                kv2 = attn_sb.tile([128, S], F16, tag="kv2")
                kt2 = attn_sb.tile([128, S], F16, tag="kt2")
                stk0 = stkp.tile([128, S], F16, tag="stk0")
                stk1 = stkp.tile([128, S], F16, tag="stk1")
                sinv_sb = stkp.tile([1, 4 * S], F16, tag="sinv_sb")
                va = []
                if ar_mode == "half":
                    cc_a = [cc_pair("a", st) for st in range(2)]
                else:
                    one = cc_pair("a", 0)
                    cc_a = [one, one]

                for st in range(2):
                    if pend_mlp_cc[st] is not None:
                        residual_and_norm_half(pa, pend_mlp_cc[st], st)
                        pend_mlp_cc[st] = None
                    qkv_rope_half(pa, wqa, q01, q23, kv2, kt2, st)
                    vtp(pa, kv2, va, 4 * st)
                    vtp(pa, kv2, va, 4 * st + 1)
                    attn_half(pa, q01, q23, kt2, kv2, va, stk0, stk1, sinv_sb,
                              wo0, wo1, cc_a[st][0], st)
                    if not skip_cc and (ar_mode == "half" or st == 1):
                        allreduce(cc_a[st][0], cc_a[st][-1])

                if debug_stage == f"stk{l}":
                    for ti, t in enumerate((stk0, stk1)):
                        nc.sync.dma_start(y_out[ti * 128:(ti + 1) * 128, :], t[:])
                    ctx_a.close()
                    break
                ctx_a.close()

                last = (l == L - 1 and debug_stage is None)
                ctx_m = ExitStack()
                pm = ctx_m.enter_context(tc.tile_pool(name="pm", bufs=1, space="PSUM"))
                if last:
                    cc_m = [None, None]
                elif ar_mode == "half":
                    cc_m = [cc_pair("m", st) for st in range(2)]
                else:
                    onem = cc_pair("m", 0)
                    cc_m = [onem, onem]
                for st in range(2):
                    residual_and_norm_half(pm, cc_a[st][1], st)
                    prod = {}
                    gu_half(pm, l, prod, st)
                    down_half(pm, l, prod, None if last else cc_m[st][0], st, last)
                    if not last and not skip_cc and (ar_mode == "half" or st == 1):
                        allreduce(cc_m[st][0], cc_m[st][-1])
                    if not last:
                        pend_mlp_cc[st] = cc_m[st][1]
                ctx_m.close()
                if debug_stage == f"attn{l}" or debug_stage == f"mlp{l}":
                    for k in range(KT):
                        nc.sync.dma_start(y_out[k * 128:(k + 1) * 128, :], XT(k))
                    break

    nc.compile()
    return nc


def make_inputs(input_ids, embed, wq, wk, wv, wo, wgate, wup, wdown, ln1, ln2):
    """host-side prep: embedding gather, layer0-norm, shard + fold gains."""
    f32 = np.float32
    f16 = np.float16
    x = np.asarray(embed, f32)[np.asarray(input_ids)[0]]      # (S, DM)
    xt = np.ascontiguousarray(x.T)                            # (DM, S)
    r0 = 1.0 / np.sqrt(np.mean(xt * xt, axis=0) + EPS)        # (S,)
    xbn0 = (xt * r0[None, :]).astype(f16)

    inv_freq = 1.0 / (THETA ** (np.arange(0, D, 2, dtype=f32) / D))
    freqs = np.arange(S, dtype=f32)[:, None] * inv_freq[None, :]    # (S, 32)
    emb = np.concatenate([freqs, freqs], axis=1)                    # (S, D)
    cosT = np.cos(emb).T.astype(f32)                                # (D, S)
    sinT = np.sin(emb).T.astype(f32)
    sinT_signed = sinT.copy()
    sinT_signed[: D // 2] *= -1.0
    cos_rep = np.concatenate([cosT, cosT], axis=0).astype(f16)      # (128, S)
    sin_rep = np.concatenate([sinT_signed, sinT_signed], axis=0).astype(f16)

    step = np.zeros((2, 128, 256), f16)
    jj = np.arange(128)[:, None]
    ii = np.arange(256)[None, :]
    for o in range(2):
        step[o] = (128 * o + jj > ii).astype(f16)
    negbigI = (NEGBIG * np.eye(128, dtype=f32)).astype(f16)
    ident = np.concatenate([np.eye(64), np.eye(64)], axis=0).astype(f16)
    cpack = np.concatenate([step[0], step[1], negbigI, ident], axis=1).astype(f16)

    scale_q = 1.0 / math.sqrt(D)
    in_maps = []
    for c in range(NCORES):
        wqkv = np.empty((L, DM, 384), f32)
        wo_c = np.empty((L, QC, DM), f32)
        wg_c = np.empty((L, DM, FFS), f32)
        wu_c = np.empty((L, DM, FFS), f32)
        wd_c = np.empty((L, FFS, DM), f32)
        for l in range(L):
            g1 = np.asarray(ln1[l], f32)[:, None]
            g2 = np.asarray(ln2[l], f32)[:, None]
            wqkv[l, :, :QC] = np.asarray(wq[l], f32)[:, c * QC:(c + 1) * QC] * g1 * scale_q
            wqkv[l, :, QC:QC + D] = np.asarray(wk[l], f32)[:, c * D:(c + 1) * D] * g1
            wqkv[l, :, QC + D:] = np.asarray(wv[l], f32)[:, c * D:(c + 1) * D] * g1
            wo_c[l] = np.asarray(wo[l], f32)[c * QC:(c + 1) * QC, :]
            wg_c[l] = np.asarray(wgate[l], f32)[:, c * FFS:(c + 1) * FFS] * g2
            wu_c[l] = np.asarray(wup[l], f32)[:, c * FFS:(c + 1) * FFS] * g2
            wd_c[l] = np.asarray(wdown[l], f32)[c * FFS:(c + 1) * FFS, :]
        in_maps.append({
            "xt16": xt.astype(f16), "xbn0": xbn0, "wqkv": wqkv.astype(f16),
            "wo": wo_c.astype(f16), "wg": wg_c.astype(f16), "wu": wu_c.astype(f16),
            "wd": wd_c.astype(f16), "cosr": cos_rep, "sinr": sin_rep,
            "cpack": cpack,
        })
    return in_maps


_NC_CACHE = {}


def kernel(**inputs) -> np.ndarray:
    if 1 not in _NC_CACHE:
        _NC_CACHE[1] = build(reps=1)
    nc = _NC_CACHE[1]
    in_maps = make_inputs(**inputs)
    res = run_bass_kernel_spmd(nc, in_maps, list(range(NCORES)))
    y = np.zeros((DM, S), np.float64)
    for c in range(NCORES):
        y += res.results[c]["y"].astype(np.float64)
    return np.ascontiguousarray(y.T.astype(np.float32)).reshape(B, S, DM)
